# revision 1
# baseline (speedup 1.0000x reference)
"""Trainium2 Bass kernel for multi-head attention + output projection.

Problem: B=4, N=2048, D=512, H=8 heads (head_dim 64), TEMP=8.0.
  logits = (Q @ K^T) / TEMP per head; P = softmax(logits); out = P @ V
  final = concat_heads(out) @ W_comb.T + b_comb

Sharding: 8 cores = 4 batches x 2 query-halves. Each core computes a full
(1024, 512) output slab independently (keys/values replicated per batch);
no collectives. Gather = pure reshape on host. Q, K and W are passed to
each core PRE-TRANSPOSED (d-major) -- a host-side layout choice that lets
every on-chip matmul read its operands directly with large contiguous DMAs
and no on-chip transposes.

Per-core algorithm, float32r matmuls (fp32 bit layout, TensorE reduced
mode: 4x faster than fp32, ~1.5e-4 matmul rel err; inputs declared f32r so
HWDGE loads them without casts), "transposed attention" so the PV matmul
needs no transpose of the softmax matrix:
  S^T[k, q] = K_h @ Q_h^T  (stationary = K^T tile, moving = Q^T; the head
              pair packs the 128 contraction rows -> row-tiled concurrent
              matmuls at base partitions 0 / 64)
  E^T = exp(S^T / TEMP)    (ScalarE straight from PSUM, f32r out; no
              max-subtraction: logits ~ N(0,1), exp is fp32-safe)
  O^T_ext = V_ext^T @ E^T  (stationary = V tile with a ones column at index
              64+32*hh, so partition 64/96 of the PSUM accumulator becomes
              the softmax denominator; 32-aligned so DVE can slice it)
  O = O^T / denom          (per-head: reciprocal on a DMA-reshaped [64,16]
              tile -> 16 elems/lane; DMA partition-broadcast via DRAM
              scratch; one in-place tensor_mul)
  F += O_h^T.T @ W^T_h     (incremental per pair into SBUF accumulators,
              bias folded into the first pair's add)

Schedule shaping (Tile scheduler follows emission order per engine): pair
p's kt-loop carries, interleaved, the previous pair's projection (late, at
kt = 11/13/15, after the previous pair's normalization chain has surely
drained, so its PSUM slot steals land where ACT has slack) and the next
pair's loads (kt==10). The last pair's normalization broadcasts its
reciprocals with a ones-row matmul through idle PE/PSUM instead of the
DRAM round-trip.
"""

import numpy as np

import concourse.bass as bass
import concourse.mybir as mybir
from concourse.tile import TileContext

F32 = mybir.dt.float32
F32R = mybir.dt.float32r

B, N, D, H = 4, 2048, 512, 8
HEAD = 64
TEMP = 8.0
NQ = N // 2          # queries per core
NCORES = 8
NKT = N // 128       # 16 key tiles of 128
NQT = NQ // 128      # 8 query tiles of 128
NPAIR = H // 2       # 4 head pairs

# this walrus build encodes at most 1 sync-wait per instruction
_MAX_WAITS = 1


def _split_excess_waits(nc):
    """Move excess per-instruction sem-waits onto preceding NoOps."""
    n_split = 0
    for f in nc.m.functions:
        for blk in f.blocks:
            insts = blk.instructions
            i = 0
            while i < len(insts):
                inst = insts[i]
                si = getattr(inst, "sync_info", None)
                if si is not None and si.on_wait and len(si.on_wait) > _MAX_WAITS:
                    waits = list(si.on_wait)
                    si.on_wait = waits[:_MAX_WAITS]
                    extra = waits[_MAX_WAITS:]
                    new_insts = []
                    for j in range(0, len(extra), _MAX_WAITS):
                        chunk = extra[j : j + _MAX_WAITS]
                        nop = mybir.InstNoOp(
                            name=f"{inst.name}-waitsplit-{j}",
                            engine=inst.engine,
                            ins=[],
                            outs=[],
                            sync_info=mybir.SyncInfo(on_wait=chunk, on_update=[]),
                        )
                        new_insts.append(nop)
                    insts[i:i] = new_insts
                    i += len(new_insts)
                    n_split += 1
                i += 1
    return n_split


def _build():
    nc = bass.Bass()
    # q/k/w arrive pre-transposed (d-major) from the host sharding step.
    # All matmul operands are declared f32r (same bit layout as fp32) so
    # HWDGE loads them directly; the PE rounds on operand load.
    qt_d = nc.dram_tensor("qt", [D, NQ], F32R, kind="ExternalInput")
    kt_d = nc.dram_tensor("kt", [D, N], F32R, kind="ExternalInput")
    v = nc.dram_tensor("v", [N, D], F32R, kind="ExternalInput")
    wt_d = nc.dram_tensor("wt", [D, D], F32R, kind="ExternalInput")
    bvec = nc.dram_tensor("bvec", [D], F32, kind="ExternalInput")
    out = nc.dram_tensor("out", [NQ, D], F32, kind="ExternalOutput")
    recips_dram = nc.dram_tensor("recips_scratch", [H, 1024], F32, kind="Internal")

    v_r = v[:, :].rearrange("(a i) d -> i a d", i=128)  # [128, 16, 512]

    with TileContext(nc) as tc:
        with (
            tc.tile_pool(name="singles", bufs=1) as singles,
            tc.tile_pool(name="tp", bufs=2) as tp,
            tc.tile_pool(name="epool", bufs=8) as epool,
            tc.tile_pool(name="nrm", bufs=2) as nrm,
            tc.tile_pool(name="psum_s", bufs=2, space="PSUM") as psum_s,
            tc.tile_pool(name="psum_o", bufs=2, space="PSUM") as psum_o,
        ):
            bias_bc = singles.tile([128, D], F32)

            # per-head O^T + denominator: rows 0:64 = O^T (normalized in
            # place), row 64 (even head) / 96 (odd head) = denominator
            otmp = []
            wts = []    # per-head W^T tiles [64 d_in, 512 d_out]
            fsb = []    # output accumulators [128 q, 512]
            for h in range(H):
                rows = 65 if h % 2 == 0 else 97
                t = singles.tile([rows, 1024], F32R, name=f"otmp{h}", tag=f"otmp{h}")
                otmp.append(t)
                t = singles.tile([64, D], F32R, name=f"wt{h}", tag=f"wt{h}")
                wts.append(t)
            for i in range(NQT):
                t = singles.tile([128, D], F32, name=f"fsb{i}", tag=f"fsb{i}")
                fsb.append(t)

            # fp32 staging for the f32r zero/one columns of V_ext
            vstage = singles.tile([128, NKT, 33], F32)
            nc.vector.memset(vstage[:, :, 0:32], 0.0)
            nc.vector.memset(vstage[:, :, 32:33], 1.0)
            ones_f = singles.tile([1, 64], F32)
            nc.vector.memset(ones_f, 1.0)
            ones_row = singles.tile([1, 64], F32R)
            nc.gpsimd.dma_start(out=ones_row, in_=ones_f)

            # persistent double-buffered V_ext tiles; the zero/ones columns
            # are written once, the V data is re-DMA'd every pair
            vxt = {0: [], 1: []}
            for hh in range(2):
                ocol = 64 + 32 * hh
                for j in range(2):
                    vx = singles.tile(
                        [128, NKT, ocol + 1], F32R,
                        name=f"vxt{hh}_{j}", tag=f"vxt{hh}_{j}",
                    )
                    vxt[hh].append(vx)

            def emit_vxt_init(j):
                for hh in range(2):
                    ocol = 64 + 32 * hh
                    vx = vxt[hh][j]
                    if hh == 1:
                        nc.gpsimd.dma_start(
                            out=vx[:, :, 64:97], in_=vstage[:, :, 0:33]
                        )
                    else:
                        nc.gpsimd.dma_start(
                            out=vx[:, :, 64:65], in_=vstage[:, :, 32:33]
                        )

            def emit_pair_loads(p):
                """Issue DMA loads for pair p; returns (qt, kt_sb, vext)."""
                hA, hB = 2 * p, 2 * p + 1
                qt = tp.tile([128, NQ], F32R, name=f"qt{p}", tag="qt")
                nc.sync.dma_start(out=qt, in_=qt_d[p * 128 : (p + 1) * 128, :])
                kt_sb = tp.tile([128, N], F32R, name=f"ktile{p}", tag="ktile")
                nc.sync.dma_start(
                    out=kt_sb[:, 0:256], in_=kt_d[p * 128 : (p + 1) * 128, 0:256]
                )
                nc.sync.dma_start(
                    out=kt_sb[:, 256:1024], in_=kt_d[p * 128 : (p + 1) * 128, 256:1024]
                )
                vext = {}
                vA = vxt[0][p % 2]
                nc.sync.dma_start(
                    out=vA[:, :, 0:64], in_=v_r[:, :, hA * HEAD : (hA + 1) * HEAD]
                )
                vext[0] = vA
                nc.sync.dma_start(
                    out=kt_sb[:, 1024:2048],
                    in_=kt_d[p * 128 : (p + 1) * 128, 1024:2048],
                )
                vB = vxt[1][p % 2]
                nc.sync.dma_start(
                    out=vB[:, :, 0:64], in_=v_r[:, :, hB * HEAD : (hB + 1) * HEAD]
                )
                vext[1] = vB
                return qt, kt_sb, vext

            def emit_proj_part(p, tiles, pool=None, tag="ps"):
                """Accumulate pair p's head contributions into fsb[tiles]."""
                hA, hB = 2 * p, 2 * p + 1
                pool = pool or psum_s
                for i in tiles:
                    ps = pool.tile([128, 512], F32, name=f"f{p}_{i}", tag=tag)
                    nc.tensor.matmul(
                        ps,
                        lhsT=otmp[hA][0:64, i * 128 : (i + 1) * 128],
                        rhs=wts[hA],
                        start=True,
                        stop=False,
                    )
                    nc.tensor.matmul(
                        ps,
                        lhsT=otmp[hB][0:64, i * 128 : (i + 1) * 128],
                        rhs=wts[hB],
                        start=False,
                        stop=True,
                    )
                    if p == 0:
                        nc.vector.tensor_add(out=fsb[i], in0=ps, in1=bias_bc)
                    else:
                        nc.vector.tensor_add(out=fsb[i], in0=ps, in1=fsb[i])
                    if p == NPAIR - 1:
                        nc.sync.dma_start(
                            out=out[i * 128 : (i + 1) * 128, :], in_=fsb[i]
                        )

            def emit_norm_head(h, hh, o_ps, tail=False):
                """Drain one head's o_ps, reciprocal its denominator, normalize."""
                rows = 65 if hh == 0 else 97
                drow = 64 + 32 * hh
                if tail and hh == 1:
                    # ACT is idle after its last exp: drain head B there so
                    # both heads' denominator chains start in parallel
                    nc.scalar.copy(otmp[h][0:rows, :], o_ps[hh][0:rows, :])
                else:
                    nc.vector.tensor_copy(otmp[h][0:rows, :], o_ps[hh][0:rows, :])
                dsq = nrm.tile([64, 16], F32, name=f"dsq{h}", tag=f"dsq{hh}")
                nc.sync.dma_start(
                    out=dsq, in_=otmp[h][drow : drow + 1, :].bitcast(F32)
                )
                rsq = nrm.tile([64, 16], F32, name=f"rsq{h}", tag=f"rsq{hh}")
                nc.vector.reciprocal(rsq, dsq)
                if tail:
                    # PE/PSUM are idle at the tail: broadcast via a ones-row
                    # matmul instead of the DRAM round-trip (saves a DMA hop)
                    strip = nrm.tile([1, 1024], F32R, name=f"strip{h}", tag=f"st{hh}")
                    nc.gpsimd.dma_start(out=strip, in_=rsq)
                    rbp = psum_s.tile([64, 1024], F32, name=f"rbp{h}", tag="ps")
                    for qc in range(2):
                        nc.tensor.matmul(
                            rbp[:, qc * 512 : (qc + 1) * 512],
                            lhsT=ones_row,
                            rhs=strip[:, qc * 512 : (qc + 1) * 512],
                            start=True,
                            stop=True,
                        )
                    nc.vector.tensor_mul(otmp[h][0:64, :], otmp[h][0:64, :], rbp)
                    return
                nc.sync.dma_start(out=recips_dram[h : h + 1, :], in_=rsq)
                rbc = nrm.tile([64, 1024], F32, name=f"rbc{h}", tag=f"rbc{hh}")
                nc.sync.dma_start(
                    out=rbc,
                    in_=recips_dram[h : h + 1, :].partition_broadcast(64),
                )
                nc.vector.tensor_mul(otmp[h][0:64, :], otmp[h][0:64, :], rbc)

            nxt = emit_pair_loads(0)
            emit_vxt_init(0)
            for p in range(NPAIR):
                hA, hB = 2 * p, 2 * p + 1
                qt, kt_sb, vext = nxt

                o_ps = {
                    0: psum_o.tile([65, 1024], F32, name=f"o{hA}", tag="o"),
                    1: psum_o.tile([97, 1024], F32, name=f"o{hB}", tag="o"),
                }

                for kt in range(NKT):
                    if p == 0 and kt == 2:
                        nc.gpsimd.dma_start(
                            out=bias_bc, in_=bvec[:].partition_broadcast(128)
                        )
                    if p == 0 and kt == 6:
                        emit_vxt_init(1)
                    if p == 0 and kt == 8:
                        for h in range(H):
                            nc.sync.dma_start(
                                out=wts[h], in_=wt_d[h * HEAD : (h + 1) * HEAD, :]
                            )
                    if kt == 10 and p + 1 < NPAIR:
                        nxt = emit_pair_loads(p + 1)

                    if p > 0 and kt in (11, 13, 15):
                        emit_proj_part(
                            p - 1,
                            ((kt - 11) // 2, (kt - 11) // 2 + 3)
                            if kt < 15
                            else (2, 5, 6, 7),
                        )
                    for hh, h in ((0, hA), (1, hB)):
                        base = hh * 64
                        s_ps = psum_s.tile(
                            [128, 1024], F32, name=f"s{h}_{kt}", tag="ps"
                        )
                        for qc in range(2):
                            nc.tensor.matmul(
                                s_ps[:, qc * 512 : (qc + 1) * 512],
                                lhsT=kt_sb[base : base + 64, kt * 128 : (kt + 1) * 128],
                                rhs=qt[base : base + 64, qc * 512 : (qc + 1) * 512],
                                start=True,
                                stop=True,
                            )
                        e_sb = epool.tile(
                            [128, 1024], F32R, name=f"e{h}_{kt}", tag="e"
                        )
                        nc.scalar.activation(
                            e_sb,
                            s_ps,
                            mybir.ActivationFunctionType.Exp,
                            bias=0.0,
                            scale=1.0 / TEMP,
                        )
                        for qc in range(2):
                            nc.tensor.matmul(
                                o_ps[hh][:, qc * 512 : (qc + 1) * 512],
                                lhsT=vext[hh][:, kt, :],
                                rhs=e_sb[:, qc * 512 : (qc + 1) * 512],
                                start=(kt == 0),
                                stop=(kt == NKT - 1),
                            )

                tail = p == NPAIR - 1
                emit_norm_head(hA, 0, o_ps, tail=tail)
                emit_norm_head(hB, 1, o_ps, tail=tail)

            emit_proj_part(NPAIR - 1, range(NQT))

    _split_excess_waits(nc)
    return nc


_NC_CACHE = {}


def _get_nc():
    if "nc" not in _NC_CACHE:
        _NC_CACHE["nc"] = _build()
    return _NC_CACHE["nc"]


def kernel(keys, queries, values, W_comb, b_comb, _collect=None):
    from concourse.bass_utils import run_bass_kernel_spmd

    keys = np.ascontiguousarray(keys, dtype=np.float32)
    queries = np.ascontiguousarray(queries, dtype=np.float32)
    values = np.ascontiguousarray(values, dtype=np.float32)
    W_comb = np.ascontiguousarray(W_comb, dtype=np.float32)
    b_comb = np.ascontiguousarray(b_comb, dtype=np.float32)

    nc = _get_nc()
    wt_np = np.ascontiguousarray(W_comb.T)
    in_maps = []
    for c in range(NCORES):
        b, half = divmod(c, 2)
        in_maps.append(
            {
                "qt": np.ascontiguousarray(
                    queries[b, half * NQ : (half + 1) * NQ, :].T
                ),
                "kt": np.ascontiguousarray(keys[b].T),
                "v": values[b],
                "wt": wt_np,
                "bvec": b_comb,
            }
        )
    kwargs = dict(_collect) if _collect else {}
    res = run_bass_kernel_spmd(nc, in_maps, core_ids=list(range(NCORES)), **kwargs)

    full = np.empty((B, N, D), dtype=np.float32)
    for c, r in enumerate(res.results):
        b, half = divmod(c, 2)
        full[b, half * NQ : (half + 1) * NQ, :] = r["out"]
    if _collect is not None:
        return full, res
    return full



# revision 3
# speedup vs baseline: 1.0315x; 1.0315x over previous
"""Trainium2 Bass kernel for multi-head attention + output projection.

Problem: B=4, N=2048, D=512, H=8 heads (head_dim 64), TEMP=8.0.
  logits = (Q @ K^T) / TEMP per head; P = softmax(logits); out = P @ V
  final = concat_heads(out) @ W_comb.T + b_comb

Sharding: 8 cores = 4 batches x 2 query-halves. Each core computes a full
(1024, 512) output slab independently (keys/values replicated per batch);
no collectives. Gather = pure reshape on host. Q, K and W are passed to
each core PRE-TRANSPOSED (d-major) so every on-chip matmul reads its
operands directly with contiguous DMAs and no on-chip transposes.

The kernel is ACT(exp)-bound: 16.8M exponentials per core stream through
ScalarE at 1 elem/cycle/lane; everything else must hide underneath. The
design processes HEADS SERIALLY (not in pairs) which shrinks the live
PSUM working set enough to give every pipeline stage its own PSUM ring:

  bank budget (16KB/partition = 8 banks):
    S^T double buffer   2 x [128,1024] f32  = 4 banks   (ACT pacing)
    O^T accum per head  2 x [65, 512] f32   = 2 banks   (q-half tags)
    projection ring     2 x [128, 512] f32  = 2 banks

Per head h, per key-tile kt (16 x 128 keys):
  S^T[k,q] = K_h @ Q_h^T      (2 matmuls N=512, f32r)
  E^T = exp(S^T / TEMP)       (ONE [128,1024] activation from PSUM)
  O^T_ext += V_ext^T @ E^T    (2 matmuls N=512 accumulating over kt;
                               V_ext has a ones column at index 64 so
                               row 64 of O^T accumulates the softmax
                               denominator for free)
Head h-1's epilogue is interleaved into head h's kt loop at fixed kt
offsets so DVE/DMA/PE epilogue work never contends with the exp stream:
  kt=0: drain O^T+denom rows PSUM->SBUF stage (DVE)
  kt=1..3: denom -> [64,16] reshape DMA -> reciprocal -> DRAM ->
           partition-broadcast [64,1024] (all small / off critical path)
  kt=4: stage rows *= recip broadcast (DVE)
  kt=6..13: per-q-tile projection matmul (K=64) + fsb accumulate (DVE),
            bias folded into head 0's accumulate
The tail (head 7 epilogue) avoids the DRAM broadcast round-trip: drain
on ACT+DVE in parallel, reciprocal directly on the staged denom row, a
ones-row matmul broadcasts it through the idle PE/PSUM, then the 8
projection tiles pipeline (PE matmul || DVE add || store DMA).
"""

import numpy as np

import concourse.bass as bass
import concourse.mybir as mybir
from concourse.tile import TileContext

F32 = mybir.dt.float32
F32R = mybir.dt.float32r

B, N, D, H = 4, 2048, 512, 8
HEAD = 64
TEMP = 8.0
NQ = N // 2          # queries per core
NCORES = 8
NKT = N // 128       # 16 key tiles of 128
NQT = NQ // 128      # 8 query tiles of 128
NPAIR = H // 2

# this walrus build encodes at most 1 sync-wait per instruction
_MAX_WAITS = 1


def _split_excess_waits(nc):
    """Move excess per-instruction sem-waits onto preceding NoOps."""
    n_split = 0
    for f in nc.m.functions:
        for blk in f.blocks:
            insts = blk.instructions
            i = 0
            while i < len(insts):
                inst = insts[i]
                si = getattr(inst, "sync_info", None)
                if si is not None and si.on_wait and len(si.on_wait) > _MAX_WAITS:
                    waits = list(si.on_wait)
                    si.on_wait = waits[:_MAX_WAITS]
                    extra = waits[_MAX_WAITS:]
                    new_insts = []
                    for j in range(0, len(extra), _MAX_WAITS):
                        chunk = extra[j : j + _MAX_WAITS]
                        nop = mybir.InstNoOp(
                            name=f"{inst.name}-waitsplit-{j}",
                            engine=inst.engine,
                            ins=[],
                            outs=[],
                            sync_info=mybir.SyncInfo(on_wait=chunk, on_update=[]),
                        )
                        new_insts.append(nop)
                    insts[i:i] = new_insts
                    i += len(new_insts)
                    n_split += 1
                i += 1
    return n_split


def _build():
    nc = bass.Bass()
    # q/k/w arrive pre-transposed (d-major) from the host sharding step.
    # All matmul operands are declared f32r (same bit layout as fp32) so
    # HWDGE loads them directly; the PE rounds on operand load.
    qt_d = nc.dram_tensor("qt", [D, NQ], F32R, kind="ExternalInput")
    kt_d = nc.dram_tensor("kt", [D, N], F32R, kind="ExternalInput")
    v = nc.dram_tensor("v", [N, D], F32R, kind="ExternalInput")
    wt_d = nc.dram_tensor("wt", [D, D], F32R, kind="ExternalInput")
    bvec = nc.dram_tensor("bvec", [D], F32, kind="ExternalInput")
    out = nc.dram_tensor("out", [NQ, D], F32, kind="ExternalOutput")
    recips_dram = nc.dram_tensor("recips_scratch", [H, 1024], F32, kind="Internal")

    v_r = v[:, :].rearrange("(a i) d -> i a d", i=128)  # [128, 16, 512]

    with TileContext(nc) as tc:
        with (
            tc.tile_pool(name="singles", bufs=1) as singles,
            tc.tile_pool(name="tp", bufs=2) as tp,
            tc.tile_pool(name="epool", bufs=4) as epool,
            tc.tile_pool(name="nrm", bufs=2) as nrm,
            tc.tile_pool(name="psum", bufs=2, space="PSUM") as psum,
        ):
            bias_bc = singles.tile([128, D], F32)
            ones_f = singles.tile([1, 64], F32)
            nc.vector.memset(ones_f, 1.0)
            ones_row = singles.tile([1, 64], F32R)
            nc.gpsimd.dma_start(out=ones_row, in_=ones_f)

            wts = []    # per-head W^T tiles [64 d_in, 512 d_out]
            fsb = []    # output accumulators [128 q, 512]
            for h in range(H):
                t = singles.tile([64, D], F32R, name=f"wt{h}", tag=f"wt{h}")
                wts.append(t)
            for i in range(NQT):
                t = singles.tile([128, D], F32, name=f"fsb{i}", tag=f"fsb{i}")
                fsb.append(t)

            def load_pair(p, first=False):
                """DMA loads for head-pair p's K^T and Q^T; returns (qt, kt_sb)."""
                qt = tp.tile([128, NQ], F32R, name=f"qt{p}", tag="qt")
                kt_sb = tp.tile([128, N], F32R, name=f"ktile{p}", tag="ktile")
                if first:
                    # first S matmuls need kt cols 0:128 + all of qt ASAP
                    nc.sync.dma_start(
                        out=kt_sb[:, 0:128], in_=kt_d[p * 128 : (p + 1) * 128, 0:128]
                    )
                    nc.sync.dma_start(out=qt[:, 0:512], in_=qt_d[p * 128 : (p + 1) * 128, 0:512])
                    nc.sync.dma_start(out=qt[:, 512:1024], in_=qt_d[p * 128 : (p + 1) * 128, 512:1024])
                    nc.sync.dma_start(
                        out=kt_sb[:, 128:1024], in_=kt_d[p * 128 : (p + 1) * 128, 128:1024]
                    )
                    nc.sync.dma_start(
                        out=kt_sb[:, 1024:2048], in_=kt_d[p * 128 : (p + 1) * 128, 1024:2048]
                    )
                else:
                    nc.sync.dma_start(out=qt, in_=qt_d[p * 128 : (p + 1) * 128, :])
                    nc.sync.dma_start(
                        out=kt_sb[:, 0:1024], in_=kt_d[p * 128 : (p + 1) * 128, 0:1024]
                    )
                    nc.sync.dma_start(
                        out=kt_sb[:, 1024:2048], in_=kt_d[p * 128 : (p + 1) * 128, 1024:2048]
                    )
                return qt, kt_sb

            def load_vxt(h):
                """V_ext tile for head h: [128, 16, 65], ones column at 64."""
                vx = tp.tile([128, NKT, HEAD + 1], F32R, name=f"vxt{h}", tag="vxt")
                nc.vector.memset(vx[:, :, HEAD : HEAD + 1].bitcast(F32), 1.0)
                nc.sync.dma_start(
                    out=vx[:, :, 0:HEAD], in_=v_r[:, :, h * HEAD : (h + 1) * HEAD]
                )
                return vx

            # stage: drained O^T (rows 0:64) + denominator (row 64) per head
            def drain_head(h, o_q, tail=False):
                st = nrm.tile([65, 1024], F32R, name=f"stage{h}", tag="st")
                if tail:
                    nc.scalar.copy(st[0:65, 0:512], o_q[0])
                else:
                    nc.vector.tensor_copy(st[0:65, 0:512], o_q[0])
                nc.vector.tensor_copy(st[0:65, 512:1024], o_q[1])
                return st

            def norm_head(h, st):
                """Reciprocal of denom row + partition-broadcast + in-place mul."""
                dsq = nrm.tile([64, 16], F32, name=f"dsq{h}", tag="dsq")
                nc.gpsimd.dma_start(out=dsq, in_=st[64:65, :].bitcast(F32))
                rsq = nrm.tile([64, 16], F32, name=f"rsq{h}", tag="rsq")
                nc.vector.reciprocal(rsq, dsq)
                nc.gpsimd.dma_start(out=recips_dram[h : h + 1, :], in_=rsq)
                rbc = nrm.tile([64, 1024], F32, name=f"rbc{h}", tag="rbc")
                nc.gpsimd.dma_start(
                    out=rbc, in_=recips_dram[h : h + 1, :].partition_broadcast(64)
                )
                return rbc

            def proj_tile(h, st, i):
                """Project q-tile i of head h's staged output into fsb[i]."""
                ps = psum.tile([128, 512], F32, name=f"pj{h}_{i}", tag="pj")
                nc.tensor.matmul(
                    ps,
                    lhsT=st[0:64, i * 128 : (i + 1) * 128],
                    rhs=wts[h],
                    start=True,
                    stop=True,
                )
                if h == 0:
                    nc.vector.tensor_add(out=fsb[i], in0=ps, in1=bias_bc)
                else:
                    nc.vector.tensor_add(out=fsb[i], in0=ps, in1=fsb[i])
                if h == H - 1:
                    nc.sync.dma_start(out=out[i * 128 : (i + 1) * 128, :], in_=fsb[i])

            pair_tiles = load_pair(0, first=True)
            vxt = {0: load_vxt(0), 1: load_vxt(1)}
            st_prev = None
            rbc_prev = None

            for h in range(H):
                pair, half = divmod(h, 2)
                base = HEAD * half
                qt, kt_sb = pair_tiles
                vx = vxt[h % 2]

                o_q = {
                    0: psum.tile([65, 512], F32, name=f"o{h}q0", tag="o0", bufs=1),
                    1: psum.tile([65, 512], F32, name=f"o{h}q1", tag="o1", bufs=1),
                }

                for kt in range(NKT):
                    # ---- interleaved epilogue of head h-1 ----
                    if h > 0:
                        if kt == 0:
                            st_prev = drain_head(h - 1, o_prev)
                        elif kt == 1:
                            rbc_prev = norm_head(h - 1, st_prev)
                        elif kt == 4:
                            nc.vector.tensor_mul(
                                st_prev[0:64, :], st_prev[0:64, :], rbc_prev
                            )
                        elif 6 <= kt < 6 + NQT:
                            proj_tile(h - 1, st_prev, kt - 6)
                    # ---- prefetches ----
                    if h == 0 and kt == 2:
                        nc.gpsimd.dma_start(
                            out=bias_bc, in_=bvec[:].partition_broadcast(128)
                        )
                    if h == 0 and 8 <= kt < 8 + H:
                        nc.gpsimd.dma_start(
                            out=wts[kt - 8],
                            in_=wt_d[(kt - 8) * HEAD : (kt - 7) * HEAD, :],
                        )
                    if kt == 6 and h + 2 < H:
                        vxt[h % 2] = load_vxt(h + 2)
                    if kt == 10 and half == 1 and pair + 1 < NPAIR:
                        next_pair_tiles = load_pair(pair + 1)

                    # ---- the exp-paced main pipeline ----
                    s = psum.tile([128, 1024], F32, name=f"s{h}_{kt}", tag="s")
                    for qc in range(2):
                        nc.tensor.matmul(
                            s[:, qc * 512 : (qc + 1) * 512],
                            lhsT=kt_sb[base : base + 64, kt * 128 : (kt + 1) * 128],
                            rhs=qt[base : base + 64, qc * 512 : (qc + 1) * 512],
                            start=True,
                            stop=True,
                        )
                    e = epool.tile([128, 1024], F32R, name=f"e{h}_{kt}", tag="e")
                    nc.scalar.activation(
                        e,
                        s,
                        mybir.ActivationFunctionType.Exp,
                        bias=0.0,
                        scale=1.0 / TEMP,
                    )
                    for qc in range(2):
                        nc.tensor.matmul(
                            o_q[qc],
                            lhsT=vx[:, kt, :],
                            rhs=e[:, qc * 512 : (qc + 1) * 512],
                            start=(kt == 0),
                            stop=(kt == NKT - 1),
                        )

                o_prev = o_q
                if half == 1 and pair + 1 < NPAIR:
                    pair_tiles = next_pair_tiles

            # ---- tail: head 7 epilogue ----
            h = H - 1
            st = drain_head(h, o_prev, tail=True)
            rstrip = nrm.tile([1, 1024], F32, name="rstrip", tag="rstrip")
            nc.vector.reciprocal(rstrip, st[64:65, :].bitcast(F32))
            # f32r provenance hop: the verifier requires matmul operands to be
            # produced as f32r; a DMA into an f32r tile satisfies it (same bits)
            rstrip_r = nrm.tile([1, 1024], F32R, name="rstrip_r", tag="rstrip_r")
            nc.gpsimd.dma_start(out=rstrip_r, in_=rstrip)
            for qc in range(2):
                rbp = psum.tile([64, 512], F32, name=f"rbp{qc}", tag="pj")
                nc.tensor.matmul(
                    rbp,
                    lhsT=ones_row,
                    rhs=rstrip_r[:, qc * 512 : (qc + 1) * 512],
                    start=True,
                    stop=True,
                )
                nc.vector.tensor_mul(
                    st[0:64, qc * 512 : (qc + 1) * 512],
                    st[0:64, qc * 512 : (qc + 1) * 512],
                    rbp,
                )
            for i in range(NQT):
                proj_tile(h, st, i)

    _split_excess_waits(nc)
    return nc


_NC_CACHE = {}


def _get_nc():
    if "nc" not in _NC_CACHE:
        _NC_CACHE["nc"] = _build()
    return _NC_CACHE["nc"]


def kernel(keys, queries, values, W_comb, b_comb, _collect=None):
    from concourse.bass_utils import run_bass_kernel_spmd

    keys = np.ascontiguousarray(keys, dtype=np.float32)
    queries = np.ascontiguousarray(queries, dtype=np.float32)
    values = np.ascontiguousarray(values, dtype=np.float32)
    W_comb = np.ascontiguousarray(W_comb, dtype=np.float32)
    b_comb = np.ascontiguousarray(b_comb, dtype=np.float32)

    nc = _get_nc()
    wt_np = np.ascontiguousarray(W_comb.T)
    in_maps = []
    for c in range(NCORES):
        b, half = divmod(c, 2)
        in_maps.append(
            {
                "qt": np.ascontiguousarray(
                    queries[b, half * NQ : (half + 1) * NQ, :].T
                ),
                "kt": np.ascontiguousarray(keys[b].T),
                "v": values[b],
                "wt": wt_np,
                "bvec": b_comb,
            }
        )
    kwargs = dict(_collect) if _collect else {}
    res = run_bass_kernel_spmd(nc, in_maps, core_ids=list(range(NCORES)), **kwargs)

    full = np.empty((B, N, D), dtype=np.float32)
    for c, r in enumerate(res.results):
        b, half = divmod(c, 2)
        full[b, half * NQ : (half + 1) * NQ, :] = r["out"]
    if _collect is not None:
        return full, res
    return full


# revision 13
# speedup vs baseline: 1.1441x; 1.1092x over previous
"""Trainium2 Bass kernel for multi-head attention + output projection.

Problem: B=4, N=2048, D=512, H=8 heads (head_dim 64), TEMP=8.0.
  logits = (Q @ K^T) / TEMP per head; P = softmax(logits); out = P @ V
  final = concat_heads(out) @ W_comb.T + b_comb

Sharding: 8 cores = 4 batches x 2 query-halves. Each core computes a full
(1024, 512) output slab independently (keys/values replicated per batch);
no collectives. Gather = pure reshape on host. Q, K and W are passed to
each core PRE-TRANSPOSED (d-major) so every on-chip matmul reads its
operands directly with contiguous DMAs and no on-chip transposes.

The kernel is ACT(exp)-bound: 16.8M exponentials per core stream through
ScalarE at 1 elem/cycle/lane; everything else must hide underneath. The
design processes HEADS SERIALLY (not in pairs) which shrinks the live
PSUM working set enough to give every pipeline stage its own PSUM ring:

  bank budget (16KB/partition = 8 banks):
    S^T double buffer   2 x [128,1024] f32  = 4 banks   (ACT pacing)
    O^T accum per head  2 x [65, 512] f32   = 2 banks   (q-half tags)
    projection ring     2 x [128, 512] f32  = 2 banks

Per head h, per key-tile kt (16 x 128 keys):
  S^T[k,q] = K_h @ Q_h^T      (2 matmuls N=512, f32r)
  E^T = exp(S^T / TEMP)       (ONE [128,1024] activation from PSUM)
  O^T_ext += V_ext^T @ E^T    (2 matmuls N=512 accumulating over kt;
                               V_ext has a ones column at index 64 so
                               row 64 of O^T accumulates the softmax
                               denominator for free)
Head h-1's epilogue is interleaved into head h's kt loop at fixed kt
offsets so DVE/DMA/PE epilogue work never contends with the exp stream:
  kt=0: drain O^T+denom rows PSUM->SBUF stage (DVE)
  kt=1..3: denom -> [64,16] reshape DMA -> reciprocal -> DRAM ->
           partition-broadcast [64,1024] (all small / off critical path)
  kt=4: stage rows *= recip broadcast (DVE)
  kt=6..13: per-q-tile projection matmul (K=64) + fsb accumulate (DVE),
            bias folded into head 0's accumulate
The tail (head 7 epilogue) avoids the DRAM broadcast round-trip: drain
on ACT+DVE in parallel, reciprocal directly on the staged denom row, a
ones-row matmul broadcasts it through the idle PE/PSUM, then the 8
projection tiles pipeline (PE matmul || DVE add || store DMA).
"""

import numpy as np

import concourse.bass as bass
import concourse.mybir as mybir
from concourse.tile import TileContext

F32 = mybir.dt.float32
F32R = mybir.dt.float32r

B, N, D, H = 4, 2048, 512, 8
HEAD = 64
TEMP = 8.0
NQ = N // 2          # queries per core
NCORES = 8
NKT = N // 128       # 16 key tiles of 128
NQT = NQ // 128      # 8 query tiles of 128
NPAIR = H // 2

# this walrus build encodes at most 1 sync-wait per instruction
_MAX_WAITS = 1


def _split_excess_waits(nc):
    """Move excess per-instruction sem-waits onto preceding NoOps.

    A NoOp carrying an unsatisfied wait BLOCKS its engine's sequencer, so
    order matters: keep the wait most likely to fire LAST (the Activation
    pacing sem, then PE) on the real instruction, and push DMA-queue /
    DVE waits (usually long satisfied) onto the NoOps in front.
    """

    def _lateness(w):
        name = getattr(w, "ant_name", "") or ""
        if name.startswith("Activation"):
            return 2
        if name.startswith("PE"):
            return 1
        return 0

    n_split = 0
    for f in nc.m.functions:
        for blk in f.blocks:
            insts = blk.instructions
            i = 0
            while i < len(insts):
                inst = insts[i]
                si = getattr(inst, "sync_info", None)
                if si is not None and si.on_wait and len(si.on_wait) > _MAX_WAITS:
                    waits = sorted(si.on_wait, key=_lateness, reverse=True)
                    si.on_wait = waits[:_MAX_WAITS]
                    extra = waits[_MAX_WAITS:]
                    new_insts = []
                    for j in range(0, len(extra), _MAX_WAITS):
                        chunk = extra[j : j + _MAX_WAITS]
                        nop = mybir.InstNoOp(
                            name=f"{inst.name}-waitsplit-{j}",
                            engine=inst.engine,
                            ins=[],
                            outs=[],
                            sync_info=mybir.SyncInfo(on_wait=chunk, on_update=[]),
                        )
                        new_insts.append(nop)
                    insts[i:i] = new_insts
                    i += len(new_insts)
                    n_split += 1
                i += 1
    return n_split


def _build():
    nc = bass.Bass()
    # q/k/w arrive pre-transposed (d-major) from the host sharding step.
    # All matmul operands are declared f32r (same bit layout as fp32) so
    # HWDGE loads them directly; the PE rounds on operand load.
    qt_d = nc.dram_tensor("qt", [D, NQ], F32R, kind="ExternalInput")
    kt_d = nc.dram_tensor("kt", [D, N], F32R, kind="ExternalInput")
    v = nc.dram_tensor("v", [N, D], F32R, kind="ExternalInput")
    wt_d = nc.dram_tensor("wt", [D, D], F32R, kind="ExternalInput")
    bvec = nc.dram_tensor("bvec", [D], F32, kind="ExternalInput")
    out = nc.dram_tensor("out", [NQ, D], F32, kind="ExternalOutput")
    recips_dram = nc.dram_tensor("recips_scratch", [H, 1024], F32, kind="Internal")

    v_r = v[:, :].rearrange("(a i) d -> i a d", i=128)  # [128, 16, 512]

    with TileContext(nc) as tc:
        with (
            tc.tile_pool(name="singles", bufs=1) as singles,
            tc.tile_pool(name="tp", bufs=3) as tp,
            tc.tile_pool(name="epool", bufs=4) as epool,
            tc.tile_pool(name="nrm", bufs=2) as nrm,
            tc.tile_pool(name="psum", bufs=2, space="PSUM") as psum,
        ):
            bias_bc = singles.tile([128, D], F32)
            ones_f = singles.tile([1, 64], F32)
            nc.vector.memset(ones_f, 1.0)
            ones_row = singles.tile([1, 64], F32R)
            nc.gpsimd.dma_start(out=ones_row, in_=ones_f)

            wts = []    # per-head W^T tiles [64 d_in, 512 d_out]
            fsb = []    # output accumulators [128 q, 512]
            for h in range(H):
                t = singles.tile([64, D], F32R, name=f"wt{h}", tag=f"wt{h}")
                wts.append(t)
            for i in range(NQT):
                t = singles.tile([128, D], F32, name=f"fsb{i}", tag=f"fsb{i}")
                fsb.append(t)

            def load_pair(p, first=False):
                """DMA loads for head-pair p's K^T and Q^T; returns (qt, kt_sb)."""
                qt = tp.tile([128, NQ], F32R, name=f"qt{p}", tag="qt")
                kt_sb = tp.tile([128, N], F32R, name=f"ktile{p}", tag="ktile")
                if first:
                    # first S matmuls need kt cols 0:128 + all of qt ASAP
                    nc.sync.dma_start(
                        out=kt_sb[:, 0:128], in_=kt_d[p * 128 : (p + 1) * 128, 0:128]
                    )
                    nc.sync.dma_start(out=qt[:, 0:512], in_=qt_d[p * 128 : (p + 1) * 128, 0:512])
                    nc.sync.dma_start(out=qt[:, 512:1024], in_=qt_d[p * 128 : (p + 1) * 128, 512:1024])
                    nc.sync.dma_start(
                        out=kt_sb[:, 128:1024], in_=kt_d[p * 128 : (p + 1) * 128, 128:1024]
                    )
                    nc.sync.dma_start(
                        out=kt_sb[:, 1024:2048], in_=kt_d[p * 128 : (p + 1) * 128, 1024:2048]
                    )
                else:
                    nc.sync.dma_start(out=qt, in_=qt_d[p * 128 : (p + 1) * 128, :])
                    nc.sync.dma_start(
                        out=kt_sb[:, 0:1024], in_=kt_d[p * 128 : (p + 1) * 128, 0:1024]
                    )
                    nc.sync.dma_start(
                        out=kt_sb[:, 1024:2048], in_=kt_d[p * 128 : (p + 1) * 128, 1024:2048]
                    )
                return qt, kt_sb

            def load_vxt(h):
                """V_ext tile for head h: [128, 16, 65], ones column at 64."""
                vx = tp.tile(
                    [128, NKT, HEAD + 1], F32R, name=f"vxt{h}", tag="vxt", bufs=4
                )
                nc.gpsimd.memset(vx[:, :, HEAD : HEAD + 1].bitcast(F32), 1.0)
                nc.sync.dma_start(
                    out=vx[:, :, 0:HEAD], in_=v_r[:, :, h * HEAD : (h + 1) * HEAD]
                )
                return vx

            # stage: drained O^T (rows 0:64) + denominator (row 64) per head
            def drain_head(h, o_q, tail=False):
                st = nrm.tile([65, 1024], F32R, name=f"stage{h}", tag="st")
                if tail:
                    nc.scalar.copy(st[0:65, 0:512], o_q[0])
                else:
                    nc.vector.tensor_copy(st[0:65, 0:512], o_q[0])
                nc.vector.tensor_copy(st[0:65, 512:1024], o_q[1])
                return st

            def norm_recip(h, st):
                """Reciprocal of the staged denom row, produced directly as
                f32r so the PE broadcast matmul can consume it (no DMA hops)."""
                rstrip = nrm.tile([1, 1024], F32R, name=f"rstrip{h}", tag="rstrip")
                # f32r == f32 bit layout; the guard only knows it isn't f32
                with nc.allow_low_precision(reason="f32r output, same bits as f32"):
                    nc.vector.reciprocal(rstrip, st[64:65, :].bitcast(F32))
                return rstrip

            def norm_bcast_mul(st, rstrip, qc):
                """Broadcast recips across 64 partitions via a ones-row matmul
                through the idle PE, then normalize the staged rows in place."""
                rbp = psum.tile([64, 512], F32, name="rbp", tag="pj")
                nc.tensor.matmul(
                    rbp,
                    lhsT=ones_row,
                    rhs=rstrip[:, qc * 512 : (qc + 1) * 512],
                    start=True,
                    stop=True,
                )
                nc.vector.tensor_mul(
                    st[0:64, qc * 512 : (qc + 1) * 512],
                    st[0:64, qc * 512 : (qc + 1) * 512],
                    rbp,
                )

            def proj_tile(h, st, i):
                """Project q-tile i of head h's staged output into fsb[i]."""
                ps = psum.tile([128, 512], F32, name=f"pj{h}_{i}", tag="pj")
                nc.tensor.matmul(
                    ps,
                    lhsT=st[0:64, i * 128 : (i + 1) * 128],
                    rhs=wts[h],
                    start=True,
                    stop=True,
                )
                if h == 0:
                    nc.vector.tensor_add(out=fsb[i], in0=ps, in1=bias_bc)
                else:
                    nc.vector.tensor_add(out=fsb[i], in0=ps, in1=fsb[i])
                if h == H - 1:
                    nc.sync.dma_start(out=out[i * 128 : (i + 1) * 128, :], in_=fsb[i])

            pair_tiles = load_pair(0, first=True)
            vxt = {0: load_vxt(0), 1: load_vxt(1)}
            st_prev = None
            rbc_prev = None

            for h in range(H):
                pair, half = divmod(h, 2)
                base = HEAD * half
                qt, kt_sb = pair_tiles
                vx = vxt[h % 2]

                o_q = {
                    0: psum.tile([65, 512], F32, name=f"o{h}q0", tag="o0", bufs=1),
                    1: psum.tile([65, 512], F32, name=f"o{h}q1", tag="o1", bufs=1),
                }

                for kt in range(NKT):
                    # ---- interleaved epilogue of head h-1 ----
                    if h > 0:
                        if kt == 0:
                            st_prev = drain_head(h - 1, o_prev)
                        elif kt == 1:
                            rbc_prev = norm_recip(h - 1, st_prev)
                        elif kt in (2, 3):
                            norm_bcast_mul(st_prev, rbc_prev, kt - 2)
                        elif 5 <= kt < 5 + NQT:
                            proj_tile(h - 1, st_prev, kt - 5)
                    # ---- prefetches ----
                    if h == 0 and kt == 2:
                        nc.gpsimd.dma_start(
                            out=bias_bc, in_=bvec[:].partition_broadcast(128)
                        )
                    if h == 0 and 8 <= kt < 8 + H:
                        nc.gpsimd.dma_start(
                            out=wts[kt - 8],
                            in_=wt_d[(kt - 8) * HEAD : (kt - 7) * HEAD, :],
                        )
                    if kt == 6 and h + 2 < H:
                        vxt[h % 2] = load_vxt(h + 2)
                    if kt == 10 and half == 1 and pair + 1 < NPAIR:
                        next_pair_tiles = load_pair(pair + 1)

                    # ---- the exp-paced main pipeline ----
                    s = psum.tile([128, 1024], F32, name=f"s{h}_{kt}", tag="s")
                    for qc in range(2):
                        nc.tensor.matmul(
                            s[:, qc * 512 : (qc + 1) * 512],
                            lhsT=kt_sb[base : base + 64, kt * 128 : (kt + 1) * 128],
                            rhs=qt[base : base + 64, qc * 512 : (qc + 1) * 512],
                            start=True,
                            stop=True,
                        )
                    e = epool.tile([128, 1024], F32R, name=f"e{h}_{kt}", tag="e")
                    nc.scalar.activation(
                        e,
                        s,
                        mybir.ActivationFunctionType.Exp,
                        bias=0.0,
                        scale=1.0 / TEMP,
                    )
                    for qc in range(2):
                        nc.tensor.matmul(
                            o_q[qc],
                            lhsT=vx[:, kt, :],
                            rhs=e[:, qc * 512 : (qc + 1) * 512],
                            start=(kt == 0),
                            stop=(kt == NKT - 1),
                        )

                o_prev = o_q
                if half == 1 and pair + 1 < NPAIR:
                    pair_tiles = next_pair_tiles

            # ---- tail: head 7 epilogue ----
            h = H - 1
            st = drain_head(h, o_prev, tail=True)
            rstrip = norm_recip(h, st)
            for qc in range(2):
                norm_bcast_mul(st, rstrip, qc)
            for i in range(NQT):
                proj_tile(h, st, i)

    _split_excess_waits(nc)
    return nc


_NC_CACHE = {}


def _get_nc():
    if "nc" not in _NC_CACHE:
        _NC_CACHE["nc"] = _build()
    return _NC_CACHE["nc"]


def kernel(keys, queries, values, W_comb, b_comb, _collect=None):
    from concourse.bass_utils import run_bass_kernel_spmd

    keys = np.ascontiguousarray(keys, dtype=np.float32)
    queries = np.ascontiguousarray(queries, dtype=np.float32)
    values = np.ascontiguousarray(values, dtype=np.float32)
    W_comb = np.ascontiguousarray(W_comb, dtype=np.float32)
    b_comb = np.ascontiguousarray(b_comb, dtype=np.float32)

    nc = _get_nc()
    wt_np = np.ascontiguousarray(W_comb.T)
    in_maps = []
    for c in range(NCORES):
        b, half = divmod(c, 2)
        in_maps.append(
            {
                "qt": np.ascontiguousarray(
                    queries[b, half * NQ : (half + 1) * NQ, :].T
                ),
                "kt": np.ascontiguousarray(keys[b].T),
                "v": values[b],
                "wt": wt_np,
                "bvec": b_comb,
            }
        )
    kwargs = dict(_collect) if _collect else {}
    res = run_bass_kernel_spmd(nc, in_maps, core_ids=list(range(NCORES)), **kwargs)

    full = np.empty((B, N, D), dtype=np.float32)
    for c, r in enumerate(res.results):
        b, half = divmod(c, 2)
        full[b, half * NQ : (half + 1) * NQ, :] = r["out"]
    if _collect is not None:
        return full, res
    return full


# revision 32
# speedup vs baseline: 1.1633x; 1.0168x over previous
"""Trainium2 Bass kernel for multi-head attention + output projection.

Problem: B=4, N=2048, D=512, H=8 heads (head_dim 64), TEMP=8.0.
  logits = (Q @ K^T) / TEMP per head; P = softmax(logits); out = P @ V
  final = concat_heads(out) @ W_comb.T + b_comb

Sharding: 8 cores = 4 batches x 2 query-halves. Each core computes a full
(1024, 512) output slab independently (keys/values replicated per batch);
no collectives. Gather = pure reshape on host. Q, K and W are passed to
each core PRE-TRANSPOSED (d-major) so every on-chip matmul reads its
operands directly with contiguous DMAs and no on-chip transposes.

The kernel is ACT(exp)-bound: 16.8M exponentials per core stream through
ScalarE at 1 elem/cycle/lane; everything else must hide underneath. The
design processes HEADS SERIALLY (not in pairs) which shrinks the live
PSUM working set enough to give every pipeline stage its own PSUM ring:

  bank budget (16KB/partition = 8 banks):
    S^T double buffer   2 x [128,1024] f32  = 4 banks   (ACT pacing)
    O^T accum per head  2 x [65, 512] f32   = 2 banks   (q-half tags)
    projection ring     2 x [128, 512] f32  = 2 banks

Per head h, per key-tile kt (16 x 128 keys):
  S^T[k,q] = K_h @ Q_h^T      (2 matmuls N=512, f32r)
  E^T = exp(S^T / TEMP)       (ONE [128,1024] activation from PSUM)
  O^T_ext += V_ext^T @ E^T    (2 matmuls N=512 accumulating over kt;
                               V_ext has a ones column at index 64 so
                               row 64 of O^T accumulates the softmax
                               denominator for free)
Head h-1's epilogue is interleaved into head h's kt loop at fixed kt
offsets so DVE/DMA/PE epilogue work never contends with the exp stream:
  kt=0: drain O^T+denom rows PSUM->SBUF stage (DVE)
  kt=1..3: denom -> [64,16] reshape DMA -> reciprocal -> DRAM ->
           partition-broadcast [64,1024] (all small / off critical path)
  kt=4: stage rows *= recip broadcast (DVE)
  kt=6..13: per-q-tile projection matmul (K=64) + fsb accumulate (DVE),
            bias folded into head 0's accumulate
The tail (head 7 epilogue) avoids the DRAM broadcast round-trip: drain
on ACT+DVE in parallel, reciprocal directly on the staged denom row, a
ones-row matmul broadcasts it through the idle PE/PSUM, then the 8
projection tiles pipeline (PE matmul || DVE add || store DMA).
"""

import numpy as np

import concourse.bass as bass
import concourse.mybir as mybir
from concourse.tile import TileContext

F32 = mybir.dt.float32
F32R = mybir.dt.float32r

B, N, D, H = 4, 2048, 512, 8
HEAD = 64
TEMP = 8.0
NQ = N // 2          # queries per core
NCORES = 8
NKT = N // 128       # 16 key tiles of 128
NQT = NQ // 128      # 8 query tiles of 128
NPAIR = H // 2

# this walrus build encodes at most 1 sync-wait per instruction
_MAX_WAITS = 1


def _split_excess_waits(nc):
    """Move excess per-instruction sem-waits onto preceding NoOps.

    A NoOp carrying an unsatisfied wait BLOCKS its engine's sequencer, so
    order matters: keep the wait most likely to fire LAST (the Activation
    pacing sem, then PE) on the real instruction, and push DMA-queue /
    DVE waits (usually long satisfied) onto the NoOps in front.
    """

    def _lateness(w):
        name = getattr(w, "ant_name", "") or ""
        if name.startswith("Activation"):
            return 2
        if name.startswith("PE"):
            return 1
        return 0

    n_split = 0
    for f in nc.m.functions:
        for blk in f.blocks:
            insts = blk.instructions
            i = 0
            while i < len(insts):
                inst = insts[i]
                si = getattr(inst, "sync_info", None)
                if si is not None and si.on_wait and len(si.on_wait) > _MAX_WAITS:
                    waits = sorted(si.on_wait, key=_lateness, reverse=True)
                    si.on_wait = waits[:_MAX_WAITS]
                    extra = waits[_MAX_WAITS:]
                    new_insts = []
                    for j in range(0, len(extra), _MAX_WAITS):
                        chunk = extra[j : j + _MAX_WAITS]
                        nop = mybir.InstNoOp(
                            name=f"{inst.name}-waitsplit-{j}",
                            engine=inst.engine,
                            ins=[],
                            outs=[],
                            sync_info=mybir.SyncInfo(on_wait=chunk, on_update=[]),
                        )
                        new_insts.append(nop)
                    insts[i:i] = new_insts
                    i += len(new_insts)
                    n_split += 1
                i += 1
    return n_split


def _build():
    nc = bass.Bass()
    # q/k/w arrive pre-transposed (d-major) from the host sharding step.
    # All matmul operands are declared f32r (same bit layout as fp32) so
    # HWDGE loads them directly; the PE rounds on operand load.
    qt_d = nc.dram_tensor("qt", [D, NQ], F32R, kind="ExternalInput")
    kt_d = nc.dram_tensor("kt", [D, N], F32R, kind="ExternalInput")
    v = nc.dram_tensor("v", [N, D], F32R, kind="ExternalInput")
    wt_d = nc.dram_tensor("wt", [D, D], F32R, kind="ExternalInput")
    bvec = nc.dram_tensor("bvec", [D], F32, kind="ExternalInput")
    out = nc.dram_tensor("out", [NQ, D], F32, kind="ExternalOutput")
    recips_dram = nc.dram_tensor("recips_scratch", [H, 1024], F32, kind="Internal")

    v_r = v[:, :].rearrange("(a i) d -> i a d", i=128)  # [128, 16, 512]

    with TileContext(nc) as tc:
        with (
            tc.tile_pool(name="singles", bufs=1) as singles,
            tc.tile_pool(name="tp", bufs=3) as tp,
            tc.tile_pool(name="epool", bufs=4) as epool,
            tc.tile_pool(name="nrm", bufs=2) as nrm,
            tc.tile_pool(name="psum", bufs=2, space="PSUM") as psum,
        ):
            bias_bc = singles.tile([128, D], F32)
            ones_f = singles.tile([1, 64], F32)
            nc.vector.memset(ones_f, 1.0)
            ones_row = singles.tile([1, 64], F32R)
            nc.gpsimd.dma_start(out=ones_row, in_=ones_f)

            # PE warm-up: the clock-gate model runs matmuls at 1/3 speed until
            # the PE has been busy ~3us; burn that window on dummy matmuls so
            # the first real S matmuls (on the startup critical path) are warm
            warm_src = singles.tile([1, 512], F32R)
            nc.vector.memset(warm_src.bitcast(F32), 0.0)
            for w in range(8):
                wps = psum.tile([64, 512], F32, name=f"warm{w}", tag="pj")
                nc.tensor.matmul(
                    wps, lhsT=warm_src[:, 0:64], rhs=warm_src, start=True, stop=True
                )

            wts = []    # per-head W^T tiles [64 d_in, 512 d_out]
            fsb = []    # output accumulators [128 q, 512]
            for h in range(H):
                t = singles.tile([64, D], F32R, name=f"wt{h}", tag=f"wt{h}")
                wts.append(t)
            for i in range(NQT):
                t = singles.tile([128, D], F32, name=f"fsb{i}", tag=f"fsb{i}")
                fsb.append(t)

            def load_pair(p, first=False):
                """DMA loads for head-pair p's K^T and Q^T; returns (qt, kt_sb)."""
                qt = tp.tile([128, NQ], F32R, name=f"qt{p}", tag="qt")
                kt_sb = tp.tile([128, N], F32R, name=f"ktile{p}", tag="ktile")
                if first:
                    # startup: first K tile, then all of Q (the exp(kt0)
                    # critical path), then the K remainder in cadence chunks
                    nc.sync.dma_start(
                        out=kt_sb[:, 0:128], in_=kt_d[p * 128 : (p + 1) * 128, 0:128]
                    )
                    nc.sync.dma_start(
                        out=qt[:, 0:512], in_=qt_d[p * 128 : (p + 1) * 128, 0:512]
                    )
                    nc.sync.dma_start(
                        out=qt[:, 512:1024],
                        in_=qt_d[p * 128 : (p + 1) * 128, 512:1024],
                    )
                    nc.sync.dma_start(
                        out=kt_sb[:, 128:512], in_=kt_d[p * 128 : (p + 1) * 128, 128:512]
                    )
                    nc.sync.dma_start(
                        out=kt_sb[:, 512:1024], in_=kt_d[p * 128 : (p + 1) * 128, 512:1024]
                    )
                    nc.sync.dma_start(
                        out=kt_sb[:, 1024:2048], in_=kt_d[p * 128 : (p + 1) * 128, 1024:2048]
                    )
                else:
                    nc.sync.dma_start(out=qt, in_=qt_d[p * 128 : (p + 1) * 128, :])
                    nc.sync.dma_start(
                        out=kt_sb[:, 0:1024], in_=kt_d[p * 128 : (p + 1) * 128, 0:1024]
                    )
                    nc.sync.dma_start(
                        out=kt_sb[:, 1024:2048], in_=kt_d[p * 128 : (p + 1) * 128, 1024:2048]
                    )
                return qt, kt_sb

            def load_vxt(h, split=False):
                """V_ext tile for head h: [128, 16, 65], ones column at 64."""
                vx = tp.tile(
                    [128, NKT, HEAD + 1], F32R, name=f"vxt{h}", tag="vxt", bufs=4
                )
                nc.gpsimd.memset(vx[:, :, HEAD : HEAD + 1].bitcast(F32), 1.0)
                if split:
                    # first kt slices land first so PV(kt0) doesn't park long
                    nc.gpsimd.dma_start(
                        out=vx[:, 0:2, 0:HEAD],
                        in_=v_r[:, 0:2, h * HEAD : (h + 1) * HEAD],
                    )
                    nc.gpsimd.dma_start(
                        out=vx[:, 2:NKT, 0:HEAD],
                        in_=v_r[:, 2:NKT, h * HEAD : (h + 1) * HEAD],
                    )
                else:
                    nc.gpsimd.dma_start(
                        out=vx[:, :, 0:HEAD], in_=v_r[:, :, h * HEAD : (h + 1) * HEAD]
                    )
                return vx

            # stage: drained O^T (rows 0:64) + denominator (row 64) per head
            def drain_head(h, o_q, tail=False):
                st = nrm.tile([65, 1024], F32R, name=f"stage{h}", tag="st")
                if tail:
                    nc.scalar.copy(st[0:65, 0:512], o_q[0])
                else:
                    nc.vector.tensor_copy(st[0:65, 0:512], o_q[0])
                nc.vector.tensor_copy(st[0:65, 512:1024], o_q[1])
                return st

            def norm_recip(h, st):
                """Reciprocal of the staged denom row, produced directly as
                f32r so the PE broadcast matmul can consume it (no DMA hops)."""
                rstrip = nrm.tile([1, 1024], F32R, name=f"rstrip{h}", tag="rstrip")
                # f32r == f32 bit layout; the guard only knows it isn't f32
                with nc.allow_low_precision(reason="f32r output, same bits as f32"):
                    nc.vector.reciprocal(rstrip, st[64:65, :].bitcast(F32))
                return rstrip

            def norm_bcast_mul(st, rstrip, qc):
                """Broadcast recips across 64 partitions via a ones-row matmul
                through the idle PE, then normalize the staged rows in place."""
                rbp = psum.tile([64, 512], F32, name="rbp", tag="pj")
                nc.tensor.matmul(
                    rbp,
                    lhsT=ones_row,
                    rhs=rstrip[:, qc * 512 : (qc + 1) * 512],
                    start=True,
                    stop=True,
                )
                nc.vector.tensor_mul(
                    st[0:64, qc * 512 : (qc + 1) * 512],
                    st[0:64, qc * 512 : (qc + 1) * 512],
                    rbp,
                )

            def proj_tile(h, st, i, via_act=False, dma_eng=None, tag="pj"):
                """Project q-tile i of head h's staged output into fsb[i].

                via_act: GPSIMD can't touch PSUM, so to take load off DVE the
                tile detours PSUM->SBUF through the (idle) ACT engine and the
                accumulate runs SBUF-only on GPSIMD.
                """
                ps = psum.tile([128, 512], F32, name=f"pj{h}_{i}", tag=tag)
                nc.tensor.matmul(
                    ps,
                    lhsT=st[0:64, i * 128 : (i + 1) * 128],
                    rhs=wts[h],
                    start=True,
                    stop=True,
                )
                if via_act:
                    tmp = epool.tile([128, 512], F32, name=f"tm{i}", tag="tm")
                    nc.scalar.copy(tmp, ps)
                    nc.gpsimd.tensor_add(
                        out=fsb[i], in0=tmp, in1=bias_bc if h == 0 else fsb[i]
                    )
                else:
                    nc.vector.tensor_add(
                        out=fsb[i], in0=ps, in1=bias_bc if h == 0 else fsb[i]
                    )
                if h == H - 1:
                    (dma_eng or nc.sync).dma_start(
                        out=out[i * 128 : (i + 1) * 128, :], in_=fsb[i]
                    )

            pair_tiles = load_pair(0, first=True)
            vxt = {0: load_vxt(0, split=True), 1: load_vxt(1)}
            st_prev = None
            rbc_prev = None

            for h in range(H):
                pair, half = divmod(h, 2)
                base = HEAD * half
                qt, kt_sb = pair_tiles
                vx = vxt[h % 2]

                o_q = {
                    0: psum.tile([65, 512], F32, name=f"o{h}q0", tag="o0", bufs=1),
                    1: psum.tile([65, 512], F32, name=f"o{h}q1", tag="o1", bufs=1),
                }

                for kt in range(NKT):
                    # ---- interleaved epilogue of head h-1 ----
                    if h > 0:
                        if kt == 0:
                            st_prev = drain_head(h - 1, o_prev)
                        elif kt == 1:
                            rbc_prev = norm_recip(h - 1, st_prev)
                        elif kt in (2, 3):
                            norm_bcast_mul(st_prev, rbc_prev, kt - 2)
                        elif 5 <= kt < 5 + NQT:
                            proj_tile(h - 1, st_prev, kt - 5)
                    # ---- prefetches ----
                    if h == 0 and kt == 2:
                        nc.gpsimd.dma_start(
                            out=bias_bc, in_=bvec[:].partition_broadcast(128)
                        )
                    if h == 0 and 8 <= kt < 8 + H:
                        nc.gpsimd.dma_start(
                            out=wts[kt - 8],
                            in_=wt_d[(kt - 8) * HEAD : (kt - 7) * HEAD, :],
                        )
                    if kt == 6 and h + 2 < H:
                        vxt[h % 2] = load_vxt(h + 2)
                    if kt == 10 and half == 1 and pair + 1 < NPAIR:
                        next_pair_tiles = load_pair(pair + 1)

                    # ---- the exp-paced main pipeline ----
                    s = psum.tile([128, 1024], F32, name=f"s{h}_{kt}", tag="s")
                    for qc in range(2):
                        nc.tensor.matmul(
                            s[:, qc * 512 : (qc + 1) * 512],
                            lhsT=kt_sb[base : base + 64, kt * 128 : (kt + 1) * 128],
                            rhs=qt[base : base + 64, qc * 512 : (qc + 1) * 512],
                            start=True,
                            stop=True,
                        )
                    e = epool.tile([128, 1024], F32R, name=f"e{h}_{kt}", tag="e")
                    if h == 0 and kt == 0:
                        # startup: per-half exps chained to each arriving Q chunk
                        for qc in range(2):
                            nc.scalar.activation(
                                e[:, qc * 512 : (qc + 1) * 512],
                                s[:, qc * 512 : (qc + 1) * 512],
                                mybir.ActivationFunctionType.Exp,
                                bias=0.0,
                                scale=1.0 / TEMP,
                            )
                    else:
                        nc.scalar.activation(
                            e,
                            s,
                            mybir.ActivationFunctionType.Exp,
                            bias=0.0,
                            scale=1.0 / TEMP,
                        )
                    for qc in range(2):
                        nc.tensor.matmul(
                            o_q[qc],
                            lhsT=vx[:, kt, :],
                            rhs=e[:, qc * 512 : (qc + 1) * 512],
                            start=(kt == 0),
                            stop=(kt == NKT - 1),
                        )

                o_prev = o_q
                if half == 1 and pair + 1 < NPAIR:
                    pair_tiles = next_pair_tiles

            # ---- tail: head 7 epilogue, pipelined per q-half ----
            h = H - 1
            st = drain_head(h, o_prev, tail=True)
            rstrip = nrm.tile([1, 1024], F32R, name="rstrip7", tag="rstrip")
            with nc.allow_low_precision(reason="f32r output, same bits as f32"):
                nc.vector.reciprocal(
                    rstrip[:, 0:512], st[64:65, 0:512].bitcast(F32)
                )
                nc.vector.reciprocal(
                    rstrip[:, 512:1024], st[64:65, 512:1024].bitcast(F32)
                )
            norm_bcast_mul(st, rstrip, 0)
            norm_bcast_mul(st, rstrip, 1)
            # tail drain: the S banks are free now, so proj tiles alternate
            # between the "pj" and "s" PSUM rings (4 slots in flight); fsb
            # adds split DVE:GPSIMD and stores alternate the two HWDGE queues
            for i in range(NQT):
                proj_tile(
                    h,
                    st,
                    i,
                    via_act=i in (1, 4, 7),
                    dma_eng=nc.scalar if i % 2 else nc.sync,
                    tag="s" if i % 2 else "pj",
                )

    _split_excess_waits(nc)
    return nc


_NC_CACHE = {}


def _get_nc():
    if "nc" not in _NC_CACHE:
        _NC_CACHE["nc"] = _build()
    return _NC_CACHE["nc"]


def kernel(keys, queries, values, W_comb, b_comb, _collect=None):
    from concourse.bass_utils import run_bass_kernel_spmd

    keys = np.ascontiguousarray(keys, dtype=np.float32)
    queries = np.ascontiguousarray(queries, dtype=np.float32)
    values = np.ascontiguousarray(values, dtype=np.float32)
    W_comb = np.ascontiguousarray(W_comb, dtype=np.float32)
    b_comb = np.ascontiguousarray(b_comb, dtype=np.float32)

    nc = _get_nc()
    wt_np = np.ascontiguousarray(W_comb.T)
    in_maps = []
    for c in range(NCORES):
        b, half = divmod(c, 2)
        in_maps.append(
            {
                "qt": np.ascontiguousarray(
                    queries[b, half * NQ : (half + 1) * NQ, :].T
                ),
                "kt": np.ascontiguousarray(keys[b].T),
                "v": values[b],
                "wt": wt_np,
                "bvec": b_comb,
            }
        )
    kwargs = dict(_collect) if _collect else {}
    res = run_bass_kernel_spmd(nc, in_maps, core_ids=list(range(NCORES)), **kwargs)

    full = np.empty((B, N, D), dtype=np.float32)
    for c, r in enumerate(res.results):
        b, half = divmod(c, 2)
        full[b, half * NQ : (half + 1) * NQ, :] = r["out"]
    if _collect is not None:
        return full, res
    return full


# revision 37
# speedup vs baseline: 1.2542x; 1.0781x over previous
"""Trainium2 Bass kernel for multi-head attention + output projection.

Problem: B=4, N=2048, D=512, H=8 heads (head_dim 64), TEMP=8.0.
  logits = (Q @ K^T) / TEMP per head; P = softmax(logits); out = P @ V
  final = concat_heads(out) @ W_comb.T + b_comb

Sharding: 8 cores = 4 batches x 2 query-halves. Each core computes a full
(1024, 512) output slab independently (keys/values replicated per batch);
no collectives. Gather = pure reshape on host. Q, K and W are passed to
each core PRE-TRANSPOSED (d-major) so every on-chip matmul reads its
operands directly with contiguous DMAs and no on-chip transposes.

The kernel is ACT(exp)-bound: 16.8M exponentials per core stream through
ScalarE at 1 elem/cycle/lane; everything else must hide underneath. The
design processes HEADS SERIALLY (not in pairs) which shrinks the live
PSUM working set enough to give every pipeline stage its own PSUM ring:

  bank budget (16KB/partition = 8 banks):
    S^T double buffer   2 x [128,1024] f32  = 4 banks   (ACT pacing)
    O^T accum per head  2 x [65, 512] f32   = 2 banks   (q-half tags)
    projection ring     2 x [128, 512] f32  = 2 banks

Per head h, per key-tile kt (16 x 128 keys):
  S^T[k,q] = K_h @ Q_h^T      (2 matmuls N=512, f32r)
  E^T = exp(S^T / TEMP)       (ONE [128,1024] activation from PSUM)
  O^T_ext += V_ext^T @ E^T    (2 matmuls N=512 accumulating over kt;
                               V_ext has a ones column at index 64 so
                               row 64 of O^T accumulates the softmax
                               denominator for free)
Head h-1's epilogue is interleaved into head h's kt loop at fixed kt
offsets so DVE/DMA/PE epilogue work never contends with the exp stream:
  kt=0: drain O^T+denom rows PSUM->SBUF stage (DVE)
  kt=1..3: denom -> [64,16] reshape DMA -> reciprocal -> DRAM ->
           partition-broadcast [64,1024] (all small / off critical path)
  kt=4: stage rows *= recip broadcast (DVE)
  kt=6..13: per-q-tile projection matmul (K=64) + fsb accumulate (DVE),
            bias folded into head 0's accumulate
The tail (head 7 epilogue) avoids the DRAM broadcast round-trip: drain
on ACT+DVE in parallel, reciprocal directly on the staged denom row, a
ones-row matmul broadcasts it through the idle PE/PSUM, then the 8
projection tiles pipeline (PE matmul || DVE add || store DMA).
"""

import numpy as np

import concourse.bass as bass
import concourse.mybir as mybir
from concourse.tile import TileContext

F32 = mybir.dt.float32
F32R = mybir.dt.float32r

B, N, D, H = 4, 2048, 512, 8
HEAD = 64
TEMP = 8.0
NQ = N // 2          # queries per core
NCORES = 8
NKT = N // 128       # 16 key tiles of 128
NQT = NQ // 128      # 8 query tiles of 128
NPAIR = H // 2

# this walrus build encodes at most 1 sync-wait per instruction
_MAX_WAITS = 1


def _split_excess_waits(nc):
    """Move excess per-instruction sem-waits onto preceding NoOps.

    A NoOp carrying an unsatisfied wait BLOCKS its engine's sequencer, so
    order matters: keep the wait most likely to fire LAST (the Activation
    pacing sem, then PE) on the real instruction, and push DMA-queue /
    DVE waits (usually long satisfied) onto the NoOps in front.
    """

    def _lateness(w):
        name = getattr(w, "ant_name", "") or ""
        if name.startswith("Activation"):
            return 2
        if name.startswith("PE"):
            return 1
        return 0

    n_split = 0
    for f in nc.m.functions:
        for blk in f.blocks:
            insts = blk.instructions
            i = 0
            while i < len(insts):
                inst = insts[i]
                si = getattr(inst, "sync_info", None)
                if si is not None and si.on_wait and len(si.on_wait) > _MAX_WAITS:
                    waits = sorted(si.on_wait, key=_lateness, reverse=True)
                    si.on_wait = waits[:_MAX_WAITS]
                    extra = waits[_MAX_WAITS:]
                    new_insts = []
                    for j in range(0, len(extra), _MAX_WAITS):
                        chunk = extra[j : j + _MAX_WAITS]
                        nop = mybir.InstNoOp(
                            name=f"{inst.name}-waitsplit-{j}",
                            engine=inst.engine,
                            ins=[],
                            outs=[],
                            sync_info=mybir.SyncInfo(on_wait=chunk, on_update=[]),
                        )
                        new_insts.append(nop)
                    insts[i:i] = new_insts
                    i += len(new_insts)
                    n_split += 1
                i += 1
    return n_split


def _build():
    nc = bass.Bass()
    # q/k/w arrive pre-transposed (d-major) from the host sharding step.
    # All matmul operands are declared f32r (same bit layout as fp32) so
    # HWDGE loads them directly; the PE rounds on operand load.
    qt_d = nc.dram_tensor("qt", [D, NQ], F32R, kind="ExternalInput")
    kt_d = nc.dram_tensor("kt", [D, N], F32R, kind="ExternalInput")
    v = nc.dram_tensor("v", [N, D], F32R, kind="ExternalInput")
    wt_d = nc.dram_tensor("wt", [D, D], F32R, kind="ExternalInput")
    bvec = nc.dram_tensor("bvec", [D], F32, kind="ExternalInput")
    out = nc.dram_tensor("out", [NQ, D], F32, kind="ExternalOutput")
    recips_dram = nc.dram_tensor("recips_scratch", [H, 1024], F32, kind="Internal")

    v_r = v[:, :].rearrange("(a i) d -> i a d", i=128)  # [128, 16, 512]

    with TileContext(nc) as tc:
        with (
            tc.tile_pool(name="singles", bufs=1) as singles,
            tc.tile_pool(name="tp", bufs=3) as tp,
            tc.tile_pool(name="epool", bufs=6) as epool,
            tc.tile_pool(name="nrm", bufs=2) as nrm,
            tc.tile_pool(name="psum", bufs=2, space="PSUM") as psum,
        ):
            bias_bc = singles.tile([128, D], F32)
            ones_f = singles.tile([1, 64], F32)
            nc.vector.memset(ones_f, 1.0)
            ones_row = singles.tile([1, 64], F32R)
            nc.gpsimd.dma_start(out=ones_row, in_=ones_f)

            # PE warm-up: the clock-gate model runs matmuls at 1/3 speed until
            # the PE has been busy ~3us; burn that window on dummy matmuls so
            # the first real S matmuls (on the startup critical path) are warm
            warm_src = singles.tile([1, 512], F32R)
            nc.vector.memset(warm_src.bitcast(F32), 0.0)
            for w in range(8):
                wps = psum.tile([64, 512], F32, name=f"warm{w}", tag="pj")
                nc.tensor.matmul(
                    wps, lhsT=warm_src[:, 0:64], rhs=warm_src, start=True, stop=True
                )

            wts = []    # per-head W^T tiles [64 d_in, 512 d_out]
            fsb = []    # output accumulators [128 q, 512]
            for h in range(H):
                t = singles.tile([64, D], F32R, name=f"wt{h}", tag=f"wt{h}")
                wts.append(t)
            for i in range(NQT):
                t = singles.tile([128, D], F32, name=f"fsb{i}", tag=f"fsb{i}")
                fsb.append(t)

            def load_pair(p, first=False):
                """DMA loads for head-pair p's K^T and Q^T; returns (qt, kt_sb)."""
                qt = tp.tile([128, NQ], F32R, name=f"qt{p}", tag="qt")
                kt_sb = tp.tile([128, N], F32R, name=f"ktile{p}", tag="ktile")
                if first:
                    # startup: first K tile, then all of Q (the exp(kt0)
                    # critical path), then the K remainder in cadence chunks
                    nc.sync.dma_start(
                        out=kt_sb[:, 0:128], in_=kt_d[p * 128 : (p + 1) * 128, 0:128]
                    )
                    nc.sync.dma_start(
                        out=qt[:, 0:512], in_=qt_d[p * 128 : (p + 1) * 128, 0:512]
                    )
                    nc.sync.dma_start(
                        out=qt[:, 512:1024],
                        in_=qt_d[p * 128 : (p + 1) * 128, 512:1024],
                    )
                    nc.sync.dma_start(
                        out=kt_sb[:, 128:512], in_=kt_d[p * 128 : (p + 1) * 128, 128:512]
                    )
                    nc.sync.dma_start(
                        out=kt_sb[:, 512:1024], in_=kt_d[p * 128 : (p + 1) * 128, 512:1024]
                    )
                    nc.sync.dma_start(
                        out=kt_sb[:, 1024:2048], in_=kt_d[p * 128 : (p + 1) * 128, 1024:2048]
                    )
                else:
                    nc.sync.dma_start(out=qt, in_=qt_d[p * 128 : (p + 1) * 128, :])
                    nc.sync.dma_start(
                        out=kt_sb[:, 0:1024], in_=kt_d[p * 128 : (p + 1) * 128, 0:1024]
                    )
                    nc.sync.dma_start(
                        out=kt_sb[:, 1024:2048], in_=kt_d[p * 128 : (p + 1) * 128, 1024:2048]
                    )
                return qt, kt_sb

            def load_vxt(h, split=False):
                """V_ext tile for head h: [128, 16, 65], ones column at 64."""
                vx = tp.tile(
                    [128, NKT, HEAD + 1], F32R, name=f"vxt{h}", tag="vxt", bufs=4
                )
                nc.gpsimd.memset(vx[:, :, HEAD : HEAD + 1].bitcast(F32), 1.0)
                if split:
                    # first kt slices land first so PV(kt0) doesn't park long
                    nc.gpsimd.dma_start(
                        out=vx[:, 0:2, 0:HEAD],
                        in_=v_r[:, 0:2, h * HEAD : (h + 1) * HEAD],
                    )
                    nc.gpsimd.dma_start(
                        out=vx[:, 2:NKT, 0:HEAD],
                        in_=v_r[:, 2:NKT, h * HEAD : (h + 1) * HEAD],
                    )
                else:
                    nc.gpsimd.dma_start(
                        out=vx[:, :, 0:HEAD], in_=v_r[:, :, h * HEAD : (h + 1) * HEAD]
                    )
                return vx

            # stage: drained O^T (rows 0:64) + denominator (row 64) per head
            def drain_head(h, o_q, tail=False):
                st = nrm.tile([65, 1024], F32R, name=f"stage{h}", tag="st")
                if tail:
                    nc.scalar.copy(st[0:65, 0:512], o_q[0])
                else:
                    nc.vector.tensor_copy(st[0:65, 0:512], o_q[0])
                nc.vector.tensor_copy(st[0:65, 512:1024], o_q[1])
                return st

            def norm_recip(h, st):
                """Reciprocal of the staged denom row, produced directly as
                f32r so the PE broadcast matmul can consume it (no DMA hops)."""
                rstrip = nrm.tile([1, 1024], F32R, name=f"rstrip{h}", tag="rstrip")
                # f32r == f32 bit layout; the guard only knows it isn't f32
                with nc.allow_low_precision(reason="f32r output, same bits as f32"):
                    nc.vector.reciprocal(rstrip, st[64:65, :].bitcast(F32))
                return rstrip

            def norm_bcast_mul(st, rstrip, qc):
                """Broadcast recips across 64 partitions via a ones-row matmul
                through the idle PE, then normalize the staged rows in place."""
                rbp = psum.tile([64, 512], F32, name="rbp", tag="pj")
                nc.tensor.matmul(
                    rbp,
                    lhsT=ones_row,
                    rhs=rstrip[:, qc * 512 : (qc + 1) * 512],
                    start=True,
                    stop=True,
                )
                nc.vector.tensor_mul(
                    st[0:64, qc * 512 : (qc + 1) * 512],
                    st[0:64, qc * 512 : (qc + 1) * 512],
                    rbp,
                )

            def proj_tile(h, st, i, via_act=False, dma_eng=None, tag="pj"):
                """Project q-tile i of head h's staged output into fsb[i].

                via_act: GPSIMD can't touch PSUM, so to take load off DVE the
                tile detours PSUM->SBUF through the (idle) ACT engine and the
                accumulate runs SBUF-only on GPSIMD.
                """
                ps = psum.tile([128, 512], F32, name=f"pj{h}_{i}", tag=tag)
                nc.tensor.matmul(
                    ps,
                    lhsT=st[0:64, i * 128 : (i + 1) * 128],
                    rhs=wts[h],
                    start=True,
                    stop=True,
                )
                if via_act:
                    tmp = epool.tile([128, 512], F32, name=f"tm{i}", tag="tm")
                    nc.scalar.copy(tmp, ps)
                    nc.gpsimd.tensor_add(
                        out=fsb[i], in0=tmp, in1=bias_bc if h == 0 else fsb[i]
                    )
                else:
                    nc.vector.tensor_add(
                        out=fsb[i], in0=ps, in1=bias_bc if h == 0 else fsb[i]
                    )
                if h == H - 1:
                    (dma_eng or nc.sync).dma_start(
                        out=out[i * 128 : (i + 1) * 128, :], in_=fsb[i]
                    )

            pair_tiles = {0: load_pair(0, first=True)}
            vxt = {0: load_vxt(0, split=True), 1: load_vxt(1)}
            st_prev = None
            rbc_prev = None
            o_by_head = {}

            def emit_s(h, kt):
                """S^T matmuls + exp for (h, kt); returns the e tile."""
                base = HEAD * (h % 2)
                qt, kt_sb = pair_tiles[h // 2]
                s = psum.tile([128, 1024], F32, name=f"s{h}_{kt}", tag="s")
                for qc in range(2):
                    nc.tensor.matmul(
                        s[:, qc * 512 : (qc + 1) * 512],
                        lhsT=kt_sb[base : base + 64, kt * 128 : (kt + 1) * 128],
                        rhs=qt[base : base + 64, qc * 512 : (qc + 1) * 512],
                        start=True,
                        stop=True,
                    )
                e = epool.tile([128, 1024], F32R, name=f"e{h}_{kt}", tag="e")
                if h == 0 and kt == 0:
                    # startup: per-half exps chained to each arriving Q chunk
                    for qc in range(2):
                        nc.scalar.activation(
                            e[:, qc * 512 : (qc + 1) * 512],
                            s[:, qc * 512 : (qc + 1) * 512],
                            mybir.ActivationFunctionType.Exp,
                            bias=0.0,
                            scale=1.0 / TEMP,
                        )
                else:
                    nc.scalar.activation(
                        e, s, mybir.ActivationFunctionType.Exp,
                        bias=0.0, scale=1.0 / TEMP,
                    )
                return e

            def emit_pv(h, kt, e):
                if kt == 0:
                    o_by_head[h] = {
                        0: psum.tile([65, 512], F32, name=f"o{h}q0", tag="o0", bufs=1),
                        1: psum.tile([65, 512], F32, name=f"o{h}q1", tag="o1", bufs=1),
                    }
                o_q = o_by_head[h]
                vx = vxt_of[h]
                for qc in range(2):
                    nc.tensor.matmul(
                        o_q[qc],
                        lhsT=vx[:, kt, :],
                        rhs=e[:, qc * 512 : (qc + 1) * 512],
                        start=(kt == 0),
                        stop=(kt == NKT - 1),
                    )

            # software-pipelined flat loop: at step i, the PE stream gets
            # S(i+2) (then exp(i+2) on ACT) BEFORE PV(i), so the exp-feeding
            # matmuls are always OLDEST among ready instructions and win PE
            # priority the moment their PSUM slot frees
            seq = [(h, kt) for h in range(H) for kt in range(NKT)]
            vxt_of = {0: vxt[0]}
            e_tiles = {}
            for j in range(2):
                e_tiles[seq[j]] = emit_s(*seq[j])
            for i, (h, kt) in enumerate(seq):
                pair, half = divmod(h, 2)
                # ---- interleaved epilogue of head h-1 ----
                if h > 0:
                    if kt == 0:
                        st_prev = drain_head(h - 1, o_by_head.pop(h - 1))
                    elif kt == 1:
                        rbc_prev = norm_recip(h - 1, st_prev)
                    elif kt in (2, 3):
                        norm_bcast_mul(st_prev, rbc_prev, kt - 2)
                    elif 5 <= kt < 5 + NQT:
                        proj_tile(h - 1, st_prev, kt - 5)
                # ---- prefetches ----
                if h == 0 and kt == 2:
                    nc.gpsimd.dma_start(
                        out=bias_bc, in_=bvec[:].partition_broadcast(128)
                    )
                if h == 0 and 8 <= kt < 8 + H:
                    nc.gpsimd.dma_start(
                        out=wts[kt - 8],
                        in_=wt_d[(kt - 8) * HEAD : (kt - 7) * HEAD, :],
                    )
                if kt == 6 and h + 2 < H:
                    vxt[h % 2] = load_vxt(h + 2)
                if kt == 10 and half == 1 and pair + 1 < NPAIR:
                    pair_tiles[pair + 1] = load_pair(pair + 1)
                    pair_tiles.pop(pair - 1, None)

                # ---- the exp-paced main pipeline, S two steps ahead ----
                if i + 2 < len(seq):
                    e_tiles[seq[i + 2]] = emit_s(*seq[i + 2])
                    vxt_of.setdefault(seq[i + 2][0], vxt[seq[i + 2][0] % 2])
                emit_pv(h, kt, e_tiles.pop((h, kt)))

            o_prev = o_by_head.pop(H - 1)

            # ---- tail: head 7 epilogue, pipelined per q-half ----
            h = H - 1
            st = drain_head(h, o_prev, tail=True)
            rstrip = nrm.tile([1, 1024], F32R, name="rstrip7", tag="rstrip")
            with nc.allow_low_precision(reason="f32r output, same bits as f32"):
                nc.vector.reciprocal(
                    rstrip[:, 0:512], st[64:65, 0:512].bitcast(F32)
                )
                nc.vector.reciprocal(
                    rstrip[:, 512:1024], st[64:65, 512:1024].bitcast(F32)
                )
            norm_bcast_mul(st, rstrip, 0)
            norm_bcast_mul(st, rstrip, 1)
            # tail drain: the S banks are free now, so proj tiles alternate
            # between the "pj" and "s" PSUM rings (4 slots in flight); fsb
            # adds split DVE:GPSIMD and stores alternate the two HWDGE queues
            for i in range(NQT):
                proj_tile(
                    h,
                    st,
                    i,
                    via_act=i in (1, 4, 7),
                    dma_eng=nc.scalar if i % 2 else nc.sync,
                    tag="s" if i % 2 else "pj",
                )

    _split_excess_waits(nc)
    return nc


_NC_CACHE = {}


def _get_nc():
    if "nc" not in _NC_CACHE:
        _NC_CACHE["nc"] = _build()
    return _NC_CACHE["nc"]


def kernel(keys, queries, values, W_comb, b_comb, _collect=None):
    from concourse.bass_utils import run_bass_kernel_spmd

    keys = np.ascontiguousarray(keys, dtype=np.float32)
    queries = np.ascontiguousarray(queries, dtype=np.float32)
    values = np.ascontiguousarray(values, dtype=np.float32)
    W_comb = np.ascontiguousarray(W_comb, dtype=np.float32)
    b_comb = np.ascontiguousarray(b_comb, dtype=np.float32)

    nc = _get_nc()
    wt_np = np.ascontiguousarray(W_comb.T)
    in_maps = []
    for c in range(NCORES):
        b, half = divmod(c, 2)
        in_maps.append(
            {
                "qt": np.ascontiguousarray(
                    queries[b, half * NQ : (half + 1) * NQ, :].T
                ),
                "kt": np.ascontiguousarray(keys[b].T),
                "v": values[b],
                "wt": wt_np,
                "bvec": b_comb,
            }
        )
    kwargs = dict(_collect) if _collect else {}
    res = run_bass_kernel_spmd(nc, in_maps, core_ids=list(range(NCORES)), **kwargs)

    full = np.empty((B, N, D), dtype=np.float32)
    for c, r in enumerate(res.results):
        b, half = divmod(c, 2)
        full[b, half * NQ : (half + 1) * NQ, :] = r["out"]
    if _collect is not None:
        return full, res
    return full


# revision 45
# speedup vs baseline: 1.2728x; 1.0148x over previous
"""Trainium2 Bass kernel for multi-head attention + output projection.

Problem: B=4, N=2048, D=512, H=8 heads (head_dim 64), TEMP=8.0.
  logits = (Q @ K^T) / TEMP per head; P = softmax(logits); out = P @ V
  final = concat_heads(out) @ W_comb.T + b_comb

Sharding: 8 cores = 4 batches x 2 query-halves. Each core computes a full
(1024, 512) output slab independently (keys/values replicated per batch);
no collectives. Gather = pure reshape on host. Q, K and W are passed to
each core PRE-TRANSPOSED (d-major) so every on-chip matmul reads its
operands directly with contiguous DMAs and no on-chip transposes.

The kernel is ACT(exp)-bound: 16.8M exponentials per core stream through
ScalarE at 1 elem/cycle/lane; everything else must hide underneath. The
design processes HEADS SERIALLY (not in pairs) which shrinks the live
PSUM working set enough to give every pipeline stage its own PSUM ring:

  bank budget (16KB/partition = 8 banks):
    S^T double buffer   2 x [128,1024] f32  = 4 banks   (ACT pacing)
    O^T accum per head  2 x [65, 512] f32   = 2 banks   (q-half tags)
    projection ring     2 x [128, 512] f32  = 2 banks

Per head h, per key-tile kt (16 x 128 keys):
  S^T[k,q] = K_h @ Q_h^T      (2 matmuls N=512, f32r)
  E^T = exp(S^T / TEMP)       (ONE [128,1024] activation from PSUM)
  O^T_ext += V_ext^T @ E^T    (2 matmuls N=512 accumulating over kt;
                               V_ext has a ones column at index 64 so
                               row 64 of O^T accumulates the softmax
                               denominator for free)
Head h-1's epilogue is interleaved into head h's kt loop at fixed kt
offsets so DVE/DMA/PE epilogue work never contends with the exp stream:
  kt=0: drain O^T+denom rows PSUM->SBUF stage (DVE)
  kt=1..3: denom -> [64,16] reshape DMA -> reciprocal -> DRAM ->
           partition-broadcast [64,1024] (all small / off critical path)
  kt=4: stage rows *= recip broadcast (DVE)
  kt=6..13: per-q-tile projection matmul (K=64) + fsb accumulate (DVE),
            bias folded into head 0's accumulate
The tail (head 7 epilogue) avoids the DRAM broadcast round-trip: drain
on ACT+DVE in parallel, reciprocal directly on the staged denom row, a
ones-row matmul broadcasts it through the idle PE/PSUM, then the 8
projection tiles pipeline (PE matmul || DVE add || store DMA).
"""

import numpy as np

import concourse.bass as bass
import concourse.mybir as mybir
from concourse.tile import TileContext

F32 = mybir.dt.float32
F32R = mybir.dt.float32r
BF16 = mybir.dt.bfloat16

B, N, D, H = 4, 2048, 512, 8
HEAD = 64
TEMP = 8.0
NQ = N // 2          # queries per core
NCORES = 8
NKT = N // 128       # 16 key tiles of 128
NQT = NQ // 128      # 8 query tiles of 128
NPAIR = H // 2

# this walrus build encodes at most 1 sync-wait per instruction
_MAX_WAITS = 1


def _split_excess_waits(nc):
    """Move excess per-instruction sem-waits onto preceding NoOps.

    A NoOp carrying an unsatisfied wait BLOCKS its engine's sequencer, so
    order matters: keep the wait most likely to fire LAST (the Activation
    pacing sem, then PE) on the real instruction, and push DMA-queue /
    DVE waits (usually long satisfied) onto the NoOps in front.
    """

    def _lateness(w):
        name = getattr(w, "ant_name", "") or ""
        if name.startswith("Activation"):
            return 2
        if name.startswith("PE"):
            return 1
        return 0

    n_split = 0
    for f in nc.m.functions:
        for blk in f.blocks:
            insts = blk.instructions
            i = 0
            while i < len(insts):
                inst = insts[i]
                si = getattr(inst, "sync_info", None)
                if si is not None and si.on_wait and len(si.on_wait) > _MAX_WAITS:
                    waits = sorted(si.on_wait, key=_lateness, reverse=True)
                    si.on_wait = waits[:_MAX_WAITS]
                    extra = waits[_MAX_WAITS:]
                    new_insts = []
                    for j in range(0, len(extra), _MAX_WAITS):
                        chunk = extra[j : j + _MAX_WAITS]
                        nop = mybir.InstNoOp(
                            name=f"{inst.name}-waitsplit-{j}",
                            engine=inst.engine,
                            ins=[],
                            outs=[],
                            sync_info=mybir.SyncInfo(on_wait=chunk, on_update=[]),
                        )
                        new_insts.append(nop)
                    insts[i:i] = new_insts
                    i += len(new_insts)
                    n_split += 1
                i += 1
    return n_split


def _build():
    nc = bass.Bass()
    # q/k/w arrive pre-transposed (d-major) from the host sharding step.
    # All matmul operands are declared f32r (same bit layout as fp32) so
    # HWDGE loads them directly; the PE rounds on operand load.
    qt_d = nc.dram_tensor("qt", [D, NQ], F32R, kind="ExternalInput")
    kt_d = nc.dram_tensor("kt", [D, N], F32R, kind="ExternalInput")
    v = nc.dram_tensor("v", [N, D], F32R, kind="ExternalInput")
    wt_d = nc.dram_tensor("wt", [D, D], F32R, kind="ExternalInput")
    bvec = nc.dram_tensor("bvec", [D], F32, kind="ExternalInput")
    # output leaves the chip as bf16 (host upcasts): halves the serial
    # store-DMA trail at the kernel tail; the single rounding of the final
    # sum adds ~1e-3 relative error against a 2e-2 gate
    out = nc.dram_tensor("out", [NQ, D], BF16, kind="ExternalOutput")
    recips_dram = nc.dram_tensor("recips_scratch", [H, 1024], F32, kind="Internal")

    v_r = v[:, :].rearrange("(a i) d -> i a d", i=128)  # [128, 16, 512]

    with TileContext(nc) as tc:
        with (
            tc.tile_pool(name="singles", bufs=1) as singles,
            tc.tile_pool(name="tp", bufs=3) as tp,
            tc.tile_pool(name="epool", bufs=6) as epool,
            tc.tile_pool(name="nrm", bufs=2) as nrm,
            tc.tile_pool(name="psum", bufs=2, space="PSUM") as psum,
        ):
            bias_bc = singles.tile([128, D], F32)
            ones_f = singles.tile([1, 64], F32)
            nc.vector.memset(ones_f, 1.0)
            ones_row = singles.tile([1, 64], F32R)
            nc.gpsimd.dma_start(out=ones_row, in_=ones_f)

            # PE warm-up: the clock-gate model runs matmuls at 1/3 speed until
            # the PE has been busy ~3us; burn that window on dummy matmuls so
            # the first real S matmuls (on the startup critical path) are warm
            warm_src = singles.tile([1, 512], F32R)
            nc.vector.memset(warm_src.bitcast(F32), 0.0)
            for w in range(8):
                wps = psum.tile([64, 512], F32, name=f"warm{w}", tag="pj")
                nc.tensor.matmul(
                    wps, lhsT=warm_src[:, 0:64], rhs=warm_src, start=True, stop=True
                )

            wts = []    # per-head W^T tiles [64 d_in, 512 d_out]
            fsb = []    # output accumulators [128 q, 512]
            for h in range(H):
                t = singles.tile([64, D], F32R, name=f"wt{h}", tag=f"wt{h}")
                wts.append(t)
            for i in range(NQT):
                t = singles.tile([128, D], F32, name=f"fsb{i}", tag=f"fsb{i}")
                fsb.append(t)

            def load_pair(p, first=False):
                """DMA loads for head-pair p's K^T and Q^T; returns (qt, kt_sb)."""
                qt = tp.tile([128, NQ], F32R, name=f"qt{p}", tag="qt")
                kt_sb = tp.tile([128, N], F32R, name=f"ktile{p}", tag="ktile")
                if first:
                    # startup: first K tile, then all of Q (the exp(kt0)
                    # critical path), then the K remainder in cadence chunks
                    nc.sync.dma_start(
                        out=kt_sb[:, 0:128], in_=kt_d[p * 128 : (p + 1) * 128, 0:128]
                    )
                    nc.sync.dma_start(
                        out=qt[:, 0:512], in_=qt_d[p * 128 : (p + 1) * 128, 0:512]
                    )
                    nc.sync.dma_start(
                        out=qt[:, 512:1024],
                        in_=qt_d[p * 128 : (p + 1) * 128, 512:1024],
                    )
                    nc.sync.dma_start(
                        out=kt_sb[:, 128:512], in_=kt_d[p * 128 : (p + 1) * 128, 128:512]
                    )
                    nc.sync.dma_start(
                        out=kt_sb[:, 512:1024], in_=kt_d[p * 128 : (p + 1) * 128, 512:1024]
                    )
                    nc.sync.dma_start(
                        out=kt_sb[:, 1024:2048], in_=kt_d[p * 128 : (p + 1) * 128, 1024:2048]
                    )
                else:
                    nc.sync.dma_start(out=qt, in_=qt_d[p * 128 : (p + 1) * 128, :])
                    nc.sync.dma_start(
                        out=kt_sb[:, 0:1024], in_=kt_d[p * 128 : (p + 1) * 128, 0:1024]
                    )
                    nc.sync.dma_start(
                        out=kt_sb[:, 1024:2048], in_=kt_d[p * 128 : (p + 1) * 128, 1024:2048]
                    )
                return qt, kt_sb

            def load_vxt(h, split=False):
                """V_ext tile for head h: [128, 16, 65], ones column at 64."""
                vx = tp.tile(
                    [128, NKT, HEAD + 1], F32R, name=f"vxt{h}", tag="vxt", bufs=4
                )
                nc.gpsimd.memset(vx[:, :, HEAD : HEAD + 1].bitcast(F32), 1.0)
                if split:
                    # first kt slices land first so PV(kt0) doesn't park long
                    nc.gpsimd.dma_start(
                        out=vx[:, 0:2, 0:HEAD],
                        in_=v_r[:, 0:2, h * HEAD : (h + 1) * HEAD],
                    )
                    nc.gpsimd.dma_start(
                        out=vx[:, 2:NKT, 0:HEAD],
                        in_=v_r[:, 2:NKT, h * HEAD : (h + 1) * HEAD],
                    )
                else:
                    nc.gpsimd.dma_start(
                        out=vx[:, :, 0:HEAD], in_=v_r[:, :, h * HEAD : (h + 1) * HEAD]
                    )
                return vx

            # stage: drained O^T (rows 0:64) + denominator (row 64) per head
            def drain_head(h, o_q, tail=False):
                st = nrm.tile([65, 1024], F32R, name=f"stage{h}", tag="st")
                if tail:
                    # ACT is idle in the tail: both drains there, freeing DVE
                    # to start the reciprocal/normalize chain immediately
                    nc.scalar.copy(st[0:65, 0:512], o_q[0])
                    nc.scalar.copy(st[0:65, 512:1024], o_q[1])
                else:
                    nc.vector.tensor_copy(st[0:65, 0:512], o_q[0])
                    nc.vector.tensor_copy(st[0:65, 512:1024], o_q[1])
                return st

            def norm_recip(h, st):
                """Reciprocal of the staged denom row, produced directly as
                f32r so the PE broadcast matmul can consume it (no DMA hops)."""
                rstrip = nrm.tile([1, 1024], F32R, name=f"rstrip{h}", tag="rstrip")
                # f32r == f32 bit layout; the guard only knows it isn't f32
                with nc.allow_low_precision(reason="f32r output, same bits as f32"):
                    nc.vector.reciprocal(rstrip, st[64:65, :].bitcast(F32))
                return rstrip

            def norm_bcast_mul(st, rstrip, qc):
                """Broadcast recips across 64 partitions via a ones-row matmul
                through the idle PE, then normalize the staged rows in place."""
                rbp = psum.tile([64, 512], F32, name="rbp", tag="pj")
                nc.tensor.matmul(
                    rbp,
                    lhsT=ones_row,
                    rhs=rstrip[:, qc * 512 : (qc + 1) * 512],
                    start=True,
                    stop=True,
                )
                nc.vector.tensor_mul(
                    st[0:64, qc * 512 : (qc + 1) * 512],
                    st[0:64, qc * 512 : (qc + 1) * 512],
                    rbp,
                )

            def proj_tile(h, st, i, via_act=False, dma_eng=None, tag="pj"):
                """Project q-tile i of head h's staged output into fsb[i].

                via_act: GPSIMD can't touch PSUM, so to take load off DVE the
                tile detours PSUM->SBUF through the (idle) ACT engine and the
                accumulate runs SBUF-only on GPSIMD.
                """
                ps = psum.tile([128, 512], F32, name=f"pj{h}_{i}", tag=tag)
                nc.tensor.matmul(
                    ps,
                    lhsT=st[0:64, i * 128 : (i + 1) * 128],
                    rhs=wts[h],
                    start=True,
                    stop=True,
                )
                if h == H - 1:
                    # final add rounds the sum to bf16; adjacent tiles share a
                    # staging pair so the store trail is 4 DMAs instead of 8
                    if i % 2 == 0:
                        tail_pairs[i // 2] = epool.tile(
                            [128, 2, 512], BF16, name=f"f16_{i}", tag="f16", bufs=4
                        )
                    f16 = tail_pairs[i // 2]
                    if via_act:
                        tmp = epool.tile([128, 512], F32, name=f"tm{i}", tag="tm")
                        nc.scalar.copy(tmp, ps)
                        nc.gpsimd.tensor_add(out=f16[:, i % 2, :], in0=tmp, in1=fsb[i])
                    else:
                        nc.vector.tensor_add(out=f16[:, i % 2, :], in0=ps, in1=fsb[i])
                    if i % 2 == 1:
                        nc.sync.dma_start(
                            out=out[(i - 1) * 128 : (i + 1) * 128, :].rearrange(
                                "(j p) d -> p j d", p=128
                            ),
                            in_=f16,
                        )
                    return
                if via_act:
                    tmp = epool.tile([128, 512], F32, name=f"tm{i}", tag="tm")
                    nc.scalar.copy(tmp, ps)
                    nc.gpsimd.tensor_add(
                        out=fsb[i], in0=tmp, in1=bias_bc if h == 0 else fsb[i]
                    )
                else:
                    nc.vector.tensor_add(
                        out=fsb[i], in0=ps, in1=bias_bc if h == 0 else fsb[i]
                    )

            pair_tiles = {0: load_pair(0, first=True)}
            vxt = {0: load_vxt(0, split=True), 1: load_vxt(1)}
            st_prev = None
            rbc_prev = None
            o_by_head = {}
            tail_pairs = {}

            def emit_s(h, kt):
                """S^T matmuls + exp for (h, kt); returns the e tile."""
                base = HEAD * (h % 2)
                qt, kt_sb = pair_tiles[h // 2]
                s = psum.tile([128, 1024], F32, name=f"s{h}_{kt}", tag="s")
                for qc in range(2):
                    nc.tensor.matmul(
                        s[:, qc * 512 : (qc + 1) * 512],
                        lhsT=kt_sb[base : base + 64, kt * 128 : (kt + 1) * 128],
                        rhs=qt[base : base + 64, qc * 512 : (qc + 1) * 512],
                        start=True,
                        stop=True,
                    )
                e = epool.tile([128, 1024], F32R, name=f"e{h}_{kt}", tag="e")
                if h == 0 and kt == 0:
                    # startup: per-half exps chained to each arriving Q chunk
                    for qc in range(2):
                        nc.scalar.activation(
                            e[:, qc * 512 : (qc + 1) * 512],
                            s[:, qc * 512 : (qc + 1) * 512],
                            mybir.ActivationFunctionType.Exp,
                            bias=0.0,
                            scale=1.0 / TEMP,
                        )
                else:
                    nc.scalar.activation(
                        e, s, mybir.ActivationFunctionType.Exp,
                        bias=0.0, scale=1.0 / TEMP,
                    )
                return e

            def emit_pv(h, kt, e):
                if kt == 0:
                    o_by_head[h] = {
                        0: psum.tile([65, 512], F32, name=f"o{h}q0", tag="o0", bufs=1),
                        1: psum.tile([65, 512], F32, name=f"o{h}q1", tag="o1", bufs=1),
                    }
                o_q = o_by_head[h]
                vx = vxt_of[h]
                for qc in range(2):
                    nc.tensor.matmul(
                        o_q[qc],
                        lhsT=vx[:, kt, :],
                        rhs=e[:, qc * 512 : (qc + 1) * 512],
                        start=(kt == 0),
                        stop=(kt == NKT - 1),
                    )

            # software-pipelined flat loop: at step i, the PE stream gets
            # S(i+2) (then exp(i+2) on ACT) BEFORE PV(i), so the exp-feeding
            # matmuls are always OLDEST among ready instructions and win PE
            # priority the moment their PSUM slot frees
            seq = [(h, kt) for h in range(H) for kt in range(NKT)]
            vxt_of = {0: vxt[0]}
            e_tiles = {}
            for j in range(2):
                e_tiles[seq[j]] = emit_s(*seq[j])
            for i, (h, kt) in enumerate(seq):
                pair, half = divmod(h, 2)
                # ---- interleaved epilogue of head h-1 ----
                if h > 0:
                    if kt == 0:
                        st_prev = drain_head(h - 1, o_by_head.pop(h - 1))
                    elif kt == 1:
                        rbc_prev = norm_recip(h - 1, st_prev)
                    elif kt in (2, 3):
                        norm_bcast_mul(st_prev, rbc_prev, kt - 2)
                    elif 5 <= kt < 5 + NQT:
                        proj_tile(h - 1, st_prev, kt - 5)
                # ---- prefetches ----
                if h == 0 and kt == 2:
                    nc.gpsimd.dma_start(
                        out=bias_bc, in_=bvec[:].partition_broadcast(128)
                    )
                if h == 0 and 8 <= kt < 8 + H:
                    nc.gpsimd.dma_start(
                        out=wts[kt - 8],
                        in_=wt_d[(kt - 8) * HEAD : (kt - 7) * HEAD, :],
                    )
                if kt == 6 and h + 2 < H:
                    vxt[h % 2] = load_vxt(h + 2)
                if kt == 10 and half == 1 and pair + 1 < NPAIR:
                    pair_tiles[pair + 1] = load_pair(pair + 1)
                    pair_tiles.pop(pair - 1, None)

                # ---- the exp-paced main pipeline, S two steps ahead ----
                if i + 2 < len(seq):
                    e_tiles[seq[i + 2]] = emit_s(*seq[i + 2])
                    vxt_of.setdefault(seq[i + 2][0], vxt[seq[i + 2][0] % 2])
                emit_pv(h, kt, e_tiles.pop((h, kt)))

            o_prev = o_by_head.pop(H - 1)

            # ---- tail: head 7 epilogue, pipelined per q-half ----
            h = H - 1
            st = drain_head(h, o_prev, tail=True)
            rstrip = nrm.tile([1, 1024], F32R, name="rstrip7", tag="rstrip")
            with nc.allow_low_precision(reason="f32r output, same bits as f32"):
                nc.vector.reciprocal(
                    rstrip[:, 0:512], st[64:65, 0:512].bitcast(F32)
                )
                nc.vector.reciprocal(
                    rstrip[:, 512:1024], st[64:65, 512:1024].bitcast(F32)
                )
            norm_bcast_mul(st, rstrip, 0)
            norm_bcast_mul(st, rstrip, 1)
            # tail drain: the S banks are free now, so proj tiles alternate
            # between the "pj" and "s" PSUM rings (4 slots in flight); fsb
            # adds split DVE:GPSIMD and stores alternate the two HWDGE queues
            for i in range(NQT):
                proj_tile(
                    h,
                    st,
                    i,
                    via_act=i in (1, 4, 7),
                    tag="s" if i % 2 else "pj",
                )

    _split_excess_waits(nc)
    return nc


_NC_CACHE = {}


def _get_nc():
    if "nc" not in _NC_CACHE:
        _NC_CACHE["nc"] = _build()
    return _NC_CACHE["nc"]


def kernel(keys, queries, values, W_comb, b_comb, _collect=None):
    from concourse.bass_utils import run_bass_kernel_spmd

    keys = np.ascontiguousarray(keys, dtype=np.float32)
    queries = np.ascontiguousarray(queries, dtype=np.float32)
    values = np.ascontiguousarray(values, dtype=np.float32)
    W_comb = np.ascontiguousarray(W_comb, dtype=np.float32)
    b_comb = np.ascontiguousarray(b_comb, dtype=np.float32)

    nc = _get_nc()
    wt_np = np.ascontiguousarray(W_comb.T)
    in_maps = []
    for c in range(NCORES):
        b, half = divmod(c, 2)
        in_maps.append(
            {
                "qt": np.ascontiguousarray(
                    queries[b, half * NQ : (half + 1) * NQ, :].T
                ),
                "kt": np.ascontiguousarray(keys[b].T),
                "v": values[b],
                "wt": wt_np,
                "bvec": b_comb,
            }
        )
    kwargs = dict(_collect) if _collect else {}
    res = run_bass_kernel_spmd(nc, in_maps, core_ids=list(range(NCORES)), **kwargs)

    full = np.empty((B, N, D), dtype=np.float32)
    for c, r in enumerate(res.results):
        b, half = divmod(c, 2)
        full[b, half * NQ : (half + 1) * NQ, :] = np.asarray(r["out"]).astype(
            np.float32
        )
    if _collect is not None:
        return full, res
    return full
